# revision 36
# baseline (speedup 1.0000x reference)
"""FP8 dynamic-quantized linear (nn_FP8Linear) on 8 Trainium2 NeuronCores.

out = fp16((x_fp8 @ w_fp8.T) / (sx*sw)) + bias, with per-tensor dynamic
fp8-e4m3 quantization of x and weight (scale = FP8_MAX / amax).

Sharding: 2x4 tensor-parallel grid. x rows split in 2 halves (replicated
across the 4 cores of a row group); weight/bias split in 4 column slabs
(replicated across the 2 cores of a column group). Each core computes a
[M/2, N/4] output slab; the host stitches the 8 slabs (no output
collective needed). This cuts per-core fp16 loads to 24MB vs 36MB for
out_features-only sharding.

Global per-tensor amaxes (must match the reference exactly) come from a
"coverage" scheme: each core's FIRST-loaded 8MB -- a distinct quarter of
its x half (m-stripe 0 after a host-side np.roll of the rows) and a
distinct n-half of its w slab (after a host-side n-roll) -- is
abs-max-reduced as it lands in SBUF, split between the DVE and GpSimd
engines so the reduction keeps pace with the DMA. Partials land in
columns of shared accumulators (one final reduce, no combine tree). The
8 cores' partial pairs are exchanged with one tiny AllGather (15us
modeled vs 28us for AllReduce) plus a local max; the union of the 8
coverage sets is exactly x and w, so the scales are the exact global
ones and quantization matches the reference bit-for-bit (modulo the
power-of-2 trick below). The rolls also let every core run the SAME
SPMD program; the host un-rolls the output slab.

The Tile scheduler serializes DmaTranspose against collectives (they
share the DMA/XBAR path), so w is loaded in NATURAL layout (plain DMA
overlaps the collective) and transposed to k-major on the otherwise-
idle PE (matmul-transpose against an identity, fp16 through PSUM is
exact), with psum->SBUF assembly copies on DVE/Act. x coverage is
DMA-transposed before the collective; the x remainder is DMA-transposed
after the scale readback (explicit dep) so it cannot delay the
collective, and output writes are dispatched from the Pool engine so
they never head-of-line-block the SP transpose stream.

Matmuls are fp8 DoubleRow (2x PE rate, 256-deep contraction per pass);
each accumulation group is split into 512-column halves because a
matmul accumulation group must stay inside one 2KB PSUM bank (the
walrus codegen rejects wider groups). Discarded fp16 matmuls bridge the
PE p-state through the amax/collective window.

TRN fp8e4 (float8_e4m3) has max +-240 vs OCP e4m3fn's +-448, so the
device uses scale 224/amax == ref_scale/2: fp8 grids are self-similar
under powers of two, so device fp8 values are exactly half the
reference's, and the dequant multipliers absorb the factor of 4.

Modeled (TimelineSim) exec time: 149260 ns vs 279277 ns for the
previous out_features-sharded kernel (1.87x).
"""

import time

import numpy as np

import concourse.bacc as bacc
import concourse.bass as bass
import concourse.bass_isa as bass_isa
import concourse.mybir as mybir
import concourse.tile as tile
from concourse import masks
from concourse.bass import _add_dep_helper
from concourse.bass_utils import run_bass_kernel_spmd

F16 = mybir.dt.float16
F32 = mybir.dt.float32
F8 = mybir.dt.float8e4

NCORES = 8
RGRP, CGRP = 2, 4       # row groups (x halves) x col groups (w slabs)
EPS = 1e-12
# device-side quantization scale numerator: ref uses 448 (e4m3fn max); we use
# 224 so quantized values stay within TRN e4m3's +-240 normal range.
DEV_FP8_MAX = 224.0
DOUBLE_ROW = True
POOL_QUANT = False
WARMUP = 30


def build_kernel(M=4096, K=4096, NSH=1024, double_row=True,
                 pool_quant=POOL_QUANT, warmup=WARMUP, out_eng="pool",
                 deq="dve", cp_act=False, preload=True):
    """Build + compile the per-core bass program.

    Per-core shapes: x [M/2, K], w [NSH, K], out [M/2, NSH] with NSH=N/4.
    double_row: fp8 DoubleRow matmuls (2x PE throughput, ~1e-4 rel noise).
    warmup: number of discarded fp16 matmuls (gated on the last w load)
    bridging the PE p-state between the w transposes and the fp8 burst.
    pool_quant: also use the gpsimd (Pool) engine for fp16->fp8 quantize.
    """
    MH = M // RGRP            # 2048 token rows per core
    KB = K // 256             # 16 k-blocks (DoubleRow contracts 256/pass)
    NSTRIPES = 4
    SWM = MH // NSTRIPES      # 512-row m-stripes
    MCH = MH // 128           # 16 m-chunks per core
    KW = K // 4               # transfer k-width (1024)
    KCH = K // 128            # 32 k-chunks
    WNT = NSH // 128          # 8 natural w tiles
    assert MH % NSTRIPES == 0 and K % 256 == 0

    nc = bacc.Bacc("TRN2", target_bir_lowering=False, debug=False,
                   num_devices=NCORES)
    x = nc.dram_tensor("x", [MH, K], F16, kind="ExternalInput").ap()
    w = nc.dram_tensor("w", [NSH, K], F16, kind="ExternalInput").ap()
    bias = nc.dram_tensor("bias", [1, NSH], F16, kind="ExternalInput").ap()
    out = nc.dram_tensor("out", [MH, NSH], F16, kind="ExternalOutput").ap()

    # greedy engine balancers (ns/elem/partition + fixed overhead),
    # calibrated against observed TimelineSim slice durations
    cp_rate = {"v": 2.2 if cp_act else 0.72, "a": 1.0}  # psum->SBUF copies
    cp_load = {k: 0.0 for k in cp_rate}
    q_rate = {"v": 0.52, "a": 0.92}               # fp16->fp8 quantize
    if pool_quant:
        q_rate["p"] = 1.48
    q_fix = {"v": 60.0, "a": 150.0, "p": 150.0}
    # reserve DVE for dequant+bias, Act for out-DMA dispatch, Pool for smalls
    q_load = {"v": 0.0, "a": 0.0}
    if pool_quant:
        q_load["p"] = 0.0

    DVE_SHARE = 0.57          # coverage amax: DVE share vs gpsimd

    with tile.TileContext(nc) as tc:
        with (
            tc.tile_pool(name="const", bufs=1) as cpool,
            tc.tile_pool(name="redu", bufs=16) as rpool,
            tc.tile_pool(name="nat", bufs=4) as natpool,
            tc.tile_pool(name="wstg", bufs=4) as wspool,
            tc.tile_pool(name="xstg", bufs=6) as xspool,
            tc.tile_pool(name="w8", bufs=KB) as w8pool,
            tc.tile_pool(name="x8", bufs=KB + 2) as x8pool,
            tc.tile_pool(name="psum", bufs=3, space="PSUM") as ppool,
            tc.tile_pool(name="tp", bufs=2, space="PSUM") as tppool,
            tc.tile_pool(name="ot", bufs=4) as opool,
            tc.tile_pool(name="dram", bufs=2, space="DRAM") as dpool,
        ):
            # ---- constants ------------------------------------------------
            bias_row = cpool.tile([1, NSH], F16, tag="bias_row")
            nc.gpsimd.dma_start(bias_row[:], bias[:])
            bias_b = cpool.tile([128, NSH], F16, tag="bias_b")
            nc.gpsimd.partition_broadcast(bias_b[:], bias_row[:])
            ident = cpool.tile([128, 128], F16, tag="ident")
            masks.make_identity(nc, ident[:])

            # partial amaxes land in columns of shared accumulators; one
            # final reduce replaces a pairwise combine tree
            dax = rpool.tile([128, 8], F32, tag="dax")
            daw = rpool.tile([128, 8], F32, tag="daw")
            pax = rpool.tile([1, 8], F32, tag="pax")
            paw = rpool.tile([1, 8], F32, tag="paw")
            nc.gpsimd.memset(dax[:], 0.0)
            nc.gpsimd.memset(daw[:], 0.0)
            nc.gpsimd.memset(pax[:], 0.0)
            nc.gpsimd.memset(paw[:], 0.0)
            n_d = {"x": 0, "w": 0}

            def amax_of(flat_ap, free, tag):
                h = int(free * DVE_SHARE) & ~63
                da = dax if tag == "x" else daw
                pa = pax if tag == "x" else paw
                i = n_d[tag]
                n_d[tag] += 1
                nc.vector.tensor_reduce(
                    da[:, i:i + 1], flat_ap[:, 0:h],
                    axis=mybir.AxisListType.X,
                    op=mybir.AluOpType.max, apply_absolute_value=True)
                nc.gpsimd.tensor_reduce(
                    pa[:, i:i + 1], flat_ap[:, h:free],
                    axis=mybir.AxisListType.XYZWC,
                    op=mybir.AluOpType.max, apply_absolute_value=True)

            # ---- w natural loads + PE transposes into k-major wstg --------
            # Half-tiles [128 n, K/2] keep the load->transpose->reuse chain
            # fine-grained so DMA never waits on the PE. After the host
            # n-roll, tiles nt<4 are this core's distinct amax coverage.
            def cp(dst_ap, src_ap, elems):
                e = min(cp_load,
                        key=lambda k: cp_load[k] + elems * cp_rate[k])
                cp_load[e] += elems * cp_rate[e] + 250.0
                if e == "v":
                    nc.vector.tensor_copy(dst_ap, src_ap)
                else:
                    nc.scalar.activation(dst_ap, src_ap,
                                         mybir.ActivationFunctionType.Copy)

            wstg = [wspool.tile([128, 8, NSH], F16, tag="wstg",
                                name=f"wstg_{g}") for g in range(4)]
            wnat = {}

            def load_wnat(nt, h):
                nat = natpool.tile([128, K // 2], F16, tag="nat",
                                   name=f"wnat_{nt}_{h}")
                nc.sync.dma_start(
                    nat[:], w[nt * 128:(nt + 1) * 128,
                              h * (K // 2):(h + 1) * (K // 2)])
                wnat[(nt, h)] = nat
                if nt < 4:
                    amax_of(nat[:], K // 2, "w")
                for g in range(2):
                    pst = tppool.tile([128, 8, 128], F16, tag="tp",
                                      name=f"tp_{nt}_{h}_{g}")
                    for j in range(8):
                        c = 8 * g + j
                        nc.tensor.transpose(
                            pst[:, j, :], nat[:, c * 128:(c + 1) * 128],
                            ident[:])
                    cp(wstg[2 * h + g][:, 0:8, nt * 128:(nt + 1) * 128],
                       pst[:], 8 * 128)

            for nt in range(4):
                for h in range(2):
                    load_wnat(nt, h)

            # ---- x stripe-0 coverage: 4 transposed transfers [SWM, K/4] ---
            xstg = {}
            for t in range(4):
                stg = xspool.tile([128, KW // 128, SWM], F16, tag="xstg",
                                  name=f"xcov_{t}")
                nc.sync.dma_start(
                    stg[:], x[0:SWM, t * KW:(t + 1) * KW], transpose=True)
                xstg[(0, t)] = stg
                amax_of(stg[:].rearrange("p a b -> p (a b)"),
                        KW // 128 * SWM, "x")

            # ---- w rest (overlaps the collective: plain DMA) --------------
            for nt in range(4, WNT):
                for h in range(2):
                    load_wnat(nt, h)

            # ---- AllGather(concat) global amaxes --------------------------
            _hp = tc.high_priority()
            _hp.__enter__()
            amax2 = rpool.tile([128, 2], F32, tag="amax2")
            nc.vector.tensor_reduce(amax2[:, 0:1], dax[:],
                                    axis=mybir.AxisListType.X,
                                    op=mybir.AluOpType.max)
            nc.vector.tensor_reduce(amax2[:, 1:2], daw[:],
                                    axis=mybir.AxisListType.X,
                                    op=mybir.AluOpType.max)
            amax2r = rpool.tile([128, 2], F32, tag="amax2r")
            nc.gpsimd.partition_all_reduce(
                amax2r[:], amax2[:], channels=128,
                reduce_op=bass_isa.ReduceOp.max)
            p2 = rpool.tile([1, 2], F32, tag="p2")
            nc.vector.tensor_reduce(p2[:, 0:1], pax[:],
                                    axis=mybir.AxisListType.X,
                                    op=mybir.AluOpType.max)
            nc.vector.tensor_reduce(p2[:, 1:2], paw[:],
                                    axis=mybir.AxisListType.X,
                                    op=mybir.AluOpType.max)
            bin2 = rpool.tile([1, 2], F32, tag="bin2")
            nc.vector.tensor_tensor(bin2[:], amax2r[0:1, :], p2[:],
                                    op=mybir.AluOpType.max)

            bin_ = dpool.tile([1, 2], F32, name="bin_")
            bout = dpool.tile([1, 2 * NCORES], F32, name="bout")
            nc.gpsimd.dma_start(bin_[:], bin2[:])
            cc = nc.gpsimd.collective_compute(
                "AllGather", mybir.AluOpType.bypass,
                replica_groups=[list(range(NCORES))],
                ins=[bin_.opt()], outs=[bout.opt()])
            g16 = rpool.tile([1, 2 * NCORES], F32, tag="g16")
            g16_read = nc.gpsimd.dma_start(g16[:], bout[:])
            # gathered layout: [c0x, c0w, c1x, c1w, ...] -> max over cores
            gm = rpool.tile([1, 2], F32, tag="gm")
            nc.vector.tensor_reduce(
                gm[:], g16[:].rearrange("a (g t) -> a t g", t=2),
                axis=mybir.AxisListType.X, op=mybir.AluOpType.max)
            nc.vector.tensor_scalar_max(gm[:], gm[:], EPS)
            gb = rpool.tile([128, 2], F32, tag="gb")
            nc.gpsimd.partition_broadcast(gb[:], gm[:])

            # scales: s = 224/amax (quant), r = 1/s (dequant), r2 = rx*rw
            u2 = rpool.tile([128, 2], F32, tag="u2")
            nc.vector.reciprocal(u2[:], gb[:])
            s2 = rpool.tile([128, 2], F32, tag="s2")
            nc.vector.tensor_scalar_mul(s2[:], u2[:], DEV_FP8_MAX)
            inv2 = rpool.tile([128, 2], F32, tag="inv2")
            nc.vector.reciprocal(inv2[:], s2[:])
            r2 = rpool.tile([128, 1], F32, tag="r2")
            nc.vector.tensor_tensor(r2[:], inv2[:, 0:1], inv2[:, 1:2],
                                    op=mybir.AluOpType.mult)
            sx, sw = s2[:, 0:1], s2[:, 1:2]
            _hp.__exit__(None, None, None)

            # ---- x stripes 1-3: transposed loads AFTER the readback -------
            # (DmaTranspose serializes against the collective; gating these
            # on the readback keeps the collective + scales path clean.)
            for s in range(1, NSTRIPES):
                for q in range(4):
                    stg = xspool.tile([128, KW // 128, SWM], F16,
                                      tag="xstg", name=f"xstg_{s}_{q}")
                    d = nc.sync.dma_start(
                        stg[:], x[s * SWM:(s + 1) * SWM,
                                  q * KW:(q + 1) * KW],
                        transpose=True)
                    _add_dep_helper(d.ins, cc.ins, sync=True,
                                    reason="hold transposes off collective")
                    xstg[(s, q)] = stg

            # ---- PE p-state bridge: discarded fp16 matmuls ----------------
            if warmup:
                dps = ppool.tile([128, NSH], F32, tag="ps", name="dps")
                rhs = wnat[(WNT - 1, 1)][:, 0:512]
                lhsT = wnat[(WNT - 1, 1)][:, 512:640]
                for _ in range(warmup):
                    nc.tensor.matmul(dps[:, 0:512], lhsT, rhs,
                                     start=True, stop=True)

            # ---- quantize (greedy engine balance) -------------------------
            q_rate_b = dict(q_rate)
            q_load_b = dict(q_load)
            if pool_quant == "burst":
                q_rate_b["p"] = 1.48
                q_load_b["p"] = 0.0

            def quant(dst_ap, src_ap, scale_ap, elems, burst=False):
                rates = q_rate_b if burst else q_rate
                loads = q_load_b if burst else q_load
                e = min(loads,
                        key=lambda k: loads[k] + elems * rates[k])
                loads[e] += elems * rates[e] + q_fix[e]
                if e == "v":
                    nc.vector.tensor_scalar(dst_ap, src_ap, scale_ap, None,
                                            op0=mybir.AluOpType.mult)
                elif e == "a":
                    nc.scalar.activation(dst_ap, src_ap,
                                         mybir.ActivationFunctionType.Copy,
                                         scale=scale_ap)
                else:
                    nc.gpsimd.tensor_scalar(dst_ap, src_ap, scale_ap, None,
                                            op0=mybir.AluOpType.mult)

            w8 = [w8pool.tile([128, 2 * NSH], F8, tag="w8", name=f"w8_{kb}")
                  for kb in range(KB)]
            x8 = {}
            for s in range(NSTRIPES):
                for kb in range(KB):
                    x8[(s, kb)] = x8pool.tile([128, 2 * SWM], F8, tag="x8",
                                              name=f"x8_{s}_{kb}")

            def w_src(kb):
                t = kb // 4            # wstg tile (KW k each, 8 chunks)
                c = 2 * kb - 8 * t
                return wstg[t][:, c:c + 2, :].rearrange("p a b -> p (a b)")

            def x_src(s, kb):
                t = kb // 4            # xcov/xstg tile (KW k, 8 chunks)
                c = 2 * kb - 8 * t
                return xstg[(s, t)][:, c:c + 2, :].rearrange(
                    "p a b -> p (a b)")

            # first burst: interleave w8 and x8 stripe-0 in kb order so the
            # PE can accumulate (w8[kb], x8[0,kb]) pairs as they appear
            for kb in range(KB):
                quant(w8[kb][:], w_src(kb), sw, 2 * NSH, burst=True)
                quant(x8[(0, kb)][:], x_src(0, kb), sx, 2 * SWM, burst=True)
            q_load["v"] += q_load_b["v"]
            q_load["a"] += q_load_b["a"]
            if preload:
                # deq+bias land on DVE (and out dispatch on its engine)
                # during the stripe phase; bias the remaining quant splits
                q_load["v"] += 29500.0 if deq == "dve" else 19000.0
                if out_eng == "act":
                    q_load["a"] += 10000.0
                if pool_quant and out_eng == "pool":
                    q_load["p"] += 16000.0

            # ---- matmul sweep ---------------------------------------------
            for mc in range(MCH):
                s = mc // (MCH // NSTRIPES)
                lm = (mc % (MCH // NSTRIPES)) * 128
                if mc % (MCH // NSTRIPES) == 0 and s > 0:
                    for kb in range(KB):
                        quant(x8[(s, kb)][:], x_src(s, kb), sx, 2 * SWM)
                ps = ppool.tile([128, NSH], F32, tag="ps")
                if double_row:
                    for kb in range(KB):
                        lhsT = x8[(s, kb)].rearrange(
                            "p (i m) -> p i m", i=2)[:, :, lm:lm + 128]
                        rhs = w8[kb].rearrange("p (i n) -> p i n", i=2)
                        # accumulation group must stay within one 2KB PSUM
                        # bank (512 fp32): run the two n-halves separately
                        for nh in range(2):
                            nc.tensor.matmul(
                                ps[:, nh * (NSH // 2):(nh + 1) * (NSH // 2)],
                                lhsT, rhs[:, :, nh * (NSH // 2):
                                          (nh + 1) * (NSH // 2)],
                                start=(kb == 0), stop=(kb == KB - 1),
                                perf_mode=mybir.MatmulPerfMode.DoubleRow)
                else:
                    for kb in range(KB):
                        for i in range(2):
                            lhsT = x8[(s, kb)][:, i * SWM + lm:
                                               i * SWM + lm + 128]
                            rhs = w8[kb][:, i * NSH:(i + 1) * NSH]
                            nc.tensor.matmul(
                                ps[:], lhsT, rhs,
                                start=(kb == 0 and i == 0),
                                stop=(kb == KB - 1 and i == 1))
                ot = opool.tile([128, NSH], F16, tag="ot")
                if deq == "split":
                    nc.vector.tensor_scalar(ot[:, 0:NSH // 2],
                                            ps[:, 0:NSH // 2], r2[:], None,
                                            op0=mybir.AluOpType.mult)
                    nc.scalar.activation(ot[:, NSH // 2:], ps[:, NSH // 2:],
                                         mybir.ActivationFunctionType.Copy,
                                         scale=r2[:])
                elif deq == "dve" or (deq == "alt" and mc % 2 == 1):
                    nc.vector.tensor_scalar(ot[:], ps[:], r2[:], None,
                                            op0=mybir.AluOpType.mult)
                else:
                    nc.scalar.activation(ot[:], ps[:],
                                         mybir.ActivationFunctionType.Copy,
                                         scale=r2[:])
                nc.vector.tensor_tensor(ot[:], ot[:], bias_b[:],
                                        op=mybir.AluOpType.add)
                if out_eng == "act":
                    nc.scalar.dma_start(out[mc * 128:(mc + 1) * 128, :],
                                        ot[:])
                elif out_eng == "pool":
                    nc.gpsimd.dma_start(out[mc * 128:(mc + 1) * 128, :],
                                        ot[:])
                else:
                    nc.sync.dma_start(out[mc * 128:(mc + 1) * 128, :], ot[:])

    nc.compile()
    return nc


_CACHE = {}


def _get_kernel(M=4096, K=4096, NSH=None, SW=None, double_row=None):
    """NSH/SW args accepted for compatibility; config is fixed internally."""
    key = (M, K)
    if key not in _CACHE:
        _CACHE[key] = build_kernel(M, K, NSH=K // CGRP,
                                   double_row=DOUBLE_ROW)
    return _CACHE[key]


def kernel(x, weight, bias):
    M, K = x.shape
    N = weight.shape[0]
    nc = _get_kernel(M, K)
    MH, NSH = M // RGRP, N // CGRP
    SH = MH // CGRP           # x m-roll unit (x coverage distinctness)
    NR = NSH // RGRP          # w n-roll unit (w coverage distinctness)

    x = np.asarray(x)
    weight = np.asarray(weight)
    bias = np.asarray(bias)
    in_maps = []
    for core in range(NCORES):
        r, c = divmod(core, CGRP)
        xh = np.roll(x[r * MH:(r + 1) * MH], -SH * c, axis=0)
        wq = np.roll(weight[c * NSH:(c + 1) * NSH], -NR * r, axis=0)
        bq = np.roll(bias[c * NSH:(c + 1) * NSH], -NR * r)
        in_maps.append({
            "x": np.ascontiguousarray(xh),
            "w": np.ascontiguousarray(wq),
            "bias": np.ascontiguousarray(bq.reshape(1, NSH)),
        })
    # The axon terminal occasionally reports a stale NRT_EXEC_UNIT error from
    # a previous session on first use; a retry lands on a recovered device.
    last_err = None
    for _ in range(3):
        try:
            res = run_bass_kernel_spmd(nc, in_maps,
                                       core_ids=list(range(NCORES)))
            break
        except Exception as e:  # noqa: BLE001
            last_err = e
            time.sleep(2.0)
    else:
        raise last_err
    full = np.empty((M, N), dtype=np.float16)
    for core in range(NCORES):
        r, c = divmod(core, CGRP)
        o = np.asarray(res.results[core]["out"])
        o = np.roll(o, (SH * c, NR * r), axis=(0, 1))
        full[r * MH:(r + 1) * MH, c * NSH:(c + 1) * NSH] = o
    return full


# revision 39
# speedup vs baseline: 1.0104x; 1.0104x over previous
"""FP8 dynamic-quantized linear (nn_FP8Linear) on 8 Trainium2 NeuronCores.

out = fp16((x_fp8 @ w_fp8.T) / (sx*sw)) + bias, with per-tensor dynamic
fp8-e4m3 quantization of x and weight (scale = FP8_MAX / amax).

Sharding: 2x4 tensor-parallel grid. x rows split in 2 halves (replicated
across the 4 cores of a row group); weight/bias split in 4 column slabs
(replicated across the 2 cores of a column group). Each core computes a
[M/2, N/4] output slab; the host stitches the 8 slabs (no output
collective needed). This cuts per-core fp16 loads to 24MB vs 36MB for
out_features-only sharding.

Global per-tensor amaxes (must match the reference exactly) come from a
"coverage" scheme: each core's FIRST-loaded 8MB -- a distinct quarter of
its x half (m-stripe 0 after a host-side np.roll of the rows) and a
distinct n-half of its w slab (after a host-side n-roll) -- is
abs-max-reduced as it lands in SBUF, split between the DVE and GpSimd
engines so the reduction keeps pace with the DMA. Partials land in
columns of shared accumulators (one final reduce, no combine tree). The
8 cores' partial pairs are exchanged with one tiny AllGather (15us
modeled vs 28us for AllReduce) plus a local max; the union of the 8
coverage sets is exactly x and w, so the scales are the exact global
ones and quantization matches the reference bit-for-bit (modulo the
power-of-2 trick below). The rolls also let every core run the SAME
SPMD program; the host un-rolls the output slab.

The Tile scheduler serializes DmaTranspose against collectives (they
share the DMA/XBAR path), so w is loaded in NATURAL layout (plain DMA
overlaps the collective) and transposed to k-major on the otherwise-
idle PE (matmul-transpose against an identity, fp16 through PSUM is
exact), with psum->SBUF assembly copies on DVE/Act. x coverage is
DMA-transposed before the collective; the x remainder is DMA-transposed
after the scale readback (explicit dep) so it cannot delay the
collective, and output writes are dispatched from the Pool engine so
they never head-of-line-block the SP transpose stream.

Matmuls are fp8 DoubleRow (2x PE rate, 256-deep contraction per pass);
each accumulation group is split into 512-column halves because a
matmul accumulation group must stay inside one 2KB PSUM bank (the
walrus codegen rejects wider groups). Discarded fp16 matmuls bridge the
PE p-state through the amax/collective window.

TRN fp8e4 (float8_e4m3) has max +-240 vs OCP e4m3fn's +-448, so the
device uses scale 224/amax == ref_scale/2: fp8 grids are self-similar
under powers of two, so device fp8 values are exactly half the
reference's, and the dequant multipliers absorb the factor of 4.

Modeled (TimelineSim) exec time: 147729 ns vs 279277 ns for the
previous out_features-sharded kernel (1.89x).
"""

import time

import numpy as np

import concourse.bacc as bacc
import concourse.bass as bass
import concourse.bass_isa as bass_isa
import concourse.mybir as mybir
import concourse.tile as tile
from concourse import masks
from concourse.bass import _add_dep_helper
from concourse.bass_utils import run_bass_kernel_spmd

F16 = mybir.dt.float16
F32 = mybir.dt.float32
F8 = mybir.dt.float8e4

NCORES = 8
RGRP, CGRP = 2, 4       # row groups (x halves) x col groups (w slabs)
EPS = 1e-12
# device-side quantization scale numerator: ref uses 448 (e4m3fn max); we use
# 224 so quantized values stay within TRN e4m3's +-240 normal range.
DEV_FP8_MAX = 224.0
DOUBLE_ROW = True
POOL_QUANT = False
WARMUP = 30


def build_kernel(M=4096, K=4096, NSH=1024, double_row=True,
                 pool_quant=POOL_QUANT, warmup=WARMUP, out_eng="pool",
                 deq="dve", cp_act=False, preload=True, qlead=1):
    """Build + compile the per-core bass program.

    Per-core shapes: x [M/2, K], w [NSH, K], out [M/2, NSH] with NSH=N/4.
    double_row: fp8 DoubleRow matmuls (2x PE throughput, ~1e-4 rel noise).
    warmup: number of discarded fp16 matmuls (gated on the last w load)
    bridging the PE p-state between the w transposes and the fp8 burst.
    pool_quant: also use the gpsimd (Pool) engine for fp16->fp8 quantize.
    """
    MH = M // RGRP            # 2048 token rows per core
    KB = K // 256             # 16 k-blocks (DoubleRow contracts 256/pass)
    NSTRIPES = 4
    SWM = MH // NSTRIPES      # 512-row m-stripes
    MCH = MH // 128           # 16 m-chunks per core
    KW = K // 4               # transfer k-width (1024)
    KCH = K // 128            # 32 k-chunks
    WNT = NSH // 128          # 8 natural w tiles
    assert MH % NSTRIPES == 0 and K % 256 == 0

    nc = bacc.Bacc("TRN2", target_bir_lowering=False, debug=False,
                   num_devices=NCORES)
    x = nc.dram_tensor("x", [MH, K], F16, kind="ExternalInput").ap()
    w = nc.dram_tensor("w", [NSH, K], F16, kind="ExternalInput").ap()
    bias = nc.dram_tensor("bias", [1, NSH], F16, kind="ExternalInput").ap()
    out = nc.dram_tensor("out", [MH, NSH], F16, kind="ExternalOutput").ap()

    # greedy engine balancers (ns/elem/partition + fixed overhead),
    # calibrated against observed TimelineSim slice durations
    cp_rate = {"v": 2.2 if cp_act else 0.72, "a": 1.0}  # psum->SBUF copies
    cp_load = {k: 0.0 for k in cp_rate}
    q_rate = {"v": 0.52, "a": 0.92}               # fp16->fp8 quantize
    if pool_quant:
        q_rate["p"] = 1.48
    q_fix = {"v": 60.0, "a": 150.0, "p": 150.0}
    # reserve DVE for dequant+bias, Act for out-DMA dispatch, Pool for smalls
    q_load = {"v": 0.0, "a": 0.0}
    if pool_quant:
        q_load["p"] = 0.0

    DVE_SHARE = 0.57          # coverage amax: DVE share vs gpsimd

    with tile.TileContext(nc) as tc:
        with (
            tc.tile_pool(name="const", bufs=1) as cpool,
            tc.tile_pool(name="redu", bufs=16) as rpool,
            tc.tile_pool(name="nat", bufs=6) as natpool,
            tc.tile_pool(name="wstg", bufs=4) as wspool,
            tc.tile_pool(name="xstg", bufs=6) as xspool,
            tc.tile_pool(name="w8", bufs=KB) as w8pool,
            tc.tile_pool(name="x8", bufs=KB + 2) as x8pool,
            tc.tile_pool(name="psum", bufs=3, space="PSUM") as ppool,
            tc.tile_pool(name="tp", bufs=2, space="PSUM") as tppool,
            tc.tile_pool(name="ot", bufs=4) as opool,
            tc.tile_pool(name="dram", bufs=2, space="DRAM") as dpool,
        ):
            # ---- constants ------------------------------------------------
            bias_row = cpool.tile([1, NSH], F16, tag="bias_row")
            nc.gpsimd.dma_start(bias_row[:], bias[:])
            bias_b = cpool.tile([128, NSH], F16, tag="bias_b")
            nc.gpsimd.partition_broadcast(bias_b[:], bias_row[:])
            ident = cpool.tile([128, 128], F16, tag="ident")
            masks.make_identity(nc, ident[:])

            # partial amaxes land in columns of shared accumulators; one
            # final reduce replaces a pairwise combine tree
            dax = rpool.tile([128, 8], F32, tag="dax")
            daw = rpool.tile([128, 8], F32, tag="daw")
            pax = rpool.tile([1, 8], F32, tag="pax")
            paw = rpool.tile([1, 8], F32, tag="paw")
            nc.gpsimd.memset(dax[:], 0.0)
            nc.gpsimd.memset(daw[:], 0.0)
            nc.gpsimd.memset(pax[:], 0.0)
            nc.gpsimd.memset(paw[:], 0.0)
            n_d = {"x": 0, "w": 0}

            def amax_of(flat_ap, free, tag):
                h = int(free * DVE_SHARE) & ~63
                da = dax if tag == "x" else daw
                pa = pax if tag == "x" else paw
                i = n_d[tag]
                n_d[tag] += 1
                nc.vector.tensor_reduce(
                    da[:, i:i + 1], flat_ap[:, 0:h],
                    axis=mybir.AxisListType.X,
                    op=mybir.AluOpType.max, apply_absolute_value=True)
                nc.gpsimd.tensor_reduce(
                    pa[:, i:i + 1], flat_ap[:, h:free],
                    axis=mybir.AxisListType.XYZWC,
                    op=mybir.AluOpType.max, apply_absolute_value=True)

            # ---- w natural loads + PE transposes into k-major wstg --------
            # Half-tiles [128 n, K/2] keep the load->transpose->reuse chain
            # fine-grained so DMA never waits on the PE. After the host
            # n-roll, tiles nt<4 are this core's distinct amax coverage.
            def cp(dst_ap, src_ap, elems):
                e = min(cp_load,
                        key=lambda k: cp_load[k] + elems * cp_rate[k])
                cp_load[e] += elems * cp_rate[e] + 250.0
                if e == "v":
                    nc.vector.tensor_copy(dst_ap, src_ap)
                else:
                    nc.scalar.activation(dst_ap, src_ap,
                                         mybir.ActivationFunctionType.Copy)

            wstg = [wspool.tile([128, 8, NSH], F16, tag="wstg",
                                name=f"wstg_{g}") for g in range(4)]
            wnat = {}

            def load_wnat(nt, h):
                nat = natpool.tile([128, K // 2], F16, tag="nat",
                                   name=f"wnat_{nt}_{h}")
                nc.sync.dma_start(
                    nat[:], w[nt * 128:(nt + 1) * 128,
                              h * (K // 2):(h + 1) * (K // 2)])
                wnat[(nt, h)] = nat
                if nt < 4:
                    amax_of(nat[:], K // 2, "w")
                for g in range(2):
                    pst = tppool.tile([128, 8, 128], F16, tag="tp",
                                      name=f"tp_{nt}_{h}_{g}")
                    for j in range(8):
                        c = 8 * g + j
                        nc.tensor.transpose(
                            pst[:, j, :], nat[:, c * 128:(c + 1) * 128],
                            ident[:])
                    cp(wstg[2 * h + g][:, 0:8, nt * 128:(nt + 1) * 128],
                       pst[:], 8 * 128)

            for nt in range(4):
                for h in range(2):
                    load_wnat(nt, h)

            # ---- x stripe-0 coverage: natural half-tiles + PE transpose ---
            # (plain DMA keeps the collective window free of DmaTranspose)
            xstg = {}
            for t in range(4):
                xstg[(0, t)] = xspool.tile([128, KW // 128, SWM], F16,
                                           tag="xstg", name=f"xcov_{t}")
            for mt in range(SWM // 128):
                for h in range(2):
                    nat = natpool.tile([128, K // 2], F16, tag="nat",
                                       name=f"xnat_{mt}_{h}")
                    nc.sync.dma_start(
                        nat[:], x[mt * 128:(mt + 1) * 128,
                                  h * (K // 2):(h + 1) * (K // 2)])
                    amax_of(nat[:], K // 2, "x")
                    for g in range(2):
                        pst = tppool.tile([128, 8, 128], F16, tag="tp",
                                          name=f"xtp_{mt}_{h}_{g}")
                        for j in range(8):
                            c = 8 * g + j
                            nc.tensor.transpose(
                                pst[:, j, :], nat[:, c * 128:(c + 1) * 128],
                                ident[:])
                        cp(xstg[(0, 2 * h + g)][:, 0:8,
                                                mt * 128:(mt + 1) * 128],
                           pst[:], 8 * 128)

            # ---- w rest (overlaps the collective: plain DMA) --------------
            for nt in range(4, WNT):
                for h in range(2):
                    load_wnat(nt, h)

            # ---- AllGather(concat) global amaxes --------------------------
            _hp = tc.high_priority()
            _hp.__enter__()
            amax2 = rpool.tile([128, 2], F32, tag="amax2")
            nc.vector.tensor_reduce(amax2[:, 0:1], dax[:],
                                    axis=mybir.AxisListType.X,
                                    op=mybir.AluOpType.max)
            nc.vector.tensor_reduce(amax2[:, 1:2], daw[:],
                                    axis=mybir.AxisListType.X,
                                    op=mybir.AluOpType.max)
            amax2r = rpool.tile([128, 2], F32, tag="amax2r")
            nc.gpsimd.partition_all_reduce(
                amax2r[:], amax2[:], channels=128,
                reduce_op=bass_isa.ReduceOp.max)
            p2 = rpool.tile([1, 2], F32, tag="p2")
            nc.vector.tensor_reduce(p2[:, 0:1], pax[:],
                                    axis=mybir.AxisListType.X,
                                    op=mybir.AluOpType.max)
            nc.vector.tensor_reduce(p2[:, 1:2], paw[:],
                                    axis=mybir.AxisListType.X,
                                    op=mybir.AluOpType.max)
            bin2 = rpool.tile([1, 2], F32, tag="bin2")
            nc.vector.tensor_tensor(bin2[:], amax2r[0:1, :], p2[:],
                                    op=mybir.AluOpType.max)

            bin_ = dpool.tile([1, 2], F32, name="bin_")
            bout = dpool.tile([1, 2 * NCORES], F32, name="bout")
            nc.gpsimd.dma_start(bin_[:], bin2[:])
            cc = nc.gpsimd.collective_compute(
                "AllGather", mybir.AluOpType.bypass,
                replica_groups=[list(range(NCORES))],
                ins=[bin_.opt()], outs=[bout.opt()])
            g16 = rpool.tile([1, 2 * NCORES], F32, tag="g16")
            g16_read = nc.gpsimd.dma_start(g16[:], bout[:])
            # gathered layout: [c0x, c0w, c1x, c1w, ...] -> max over cores
            gm = rpool.tile([1, 2], F32, tag="gm")
            nc.vector.tensor_reduce(
                gm[:], g16[:].rearrange("a (g t) -> a t g", t=2),
                axis=mybir.AxisListType.X, op=mybir.AluOpType.max)
            nc.vector.tensor_scalar_max(gm[:], gm[:], EPS)
            gb = rpool.tile([128, 2], F32, tag="gb")
            nc.gpsimd.partition_broadcast(gb[:], gm[:])

            # scales: s = 224/amax (quant), r = 1/s (dequant), r2 = rx*rw
            u2 = rpool.tile([128, 2], F32, tag="u2")
            nc.vector.reciprocal(u2[:], gb[:])
            s2 = rpool.tile([128, 2], F32, tag="s2")
            nc.vector.tensor_scalar_mul(s2[:], u2[:], DEV_FP8_MAX)
            inv2 = rpool.tile([128, 2], F32, tag="inv2")
            nc.vector.reciprocal(inv2[:], s2[:])
            r2 = rpool.tile([128, 1], F32, tag="r2")
            nc.vector.tensor_tensor(r2[:], inv2[:, 0:1], inv2[:, 1:2],
                                    op=mybir.AluOpType.mult)
            sx, sw = s2[:, 0:1], s2[:, 1:2]
            _hp.__exit__(None, None, None)

            # ---- x stripes 1-3: transposed loads AFTER the readback -------
            # (DmaTranspose serializes against the collective; gating these
            # on the readback keeps the collective + scales path clean.)
            for s in range(1, NSTRIPES):
                for q in range(4):
                    stg = xspool.tile([128, KW // 128, SWM], F16,
                                      tag="xstg", name=f"xstg_{s}_{q}")
                    d = nc.sync.dma_start(
                        stg[:], x[s * SWM:(s + 1) * SWM,
                                  q * KW:(q + 1) * KW],
                        transpose=True)
                    _add_dep_helper(d.ins, cc.ins, sync=True,
                                    reason="hold transposes off collective")
                    xstg[(s, q)] = stg

            # ---- PE p-state bridge: discarded fp16 matmuls ----------------
            if warmup:
                dps = ppool.tile([128, NSH], F32, tag="ps", name="dps")
                rhs = wnat[(WNT - 1, 1)][:, 0:512]
                lhsT = wnat[(WNT - 1, 1)][:, 512:640]
                for _ in range(warmup):
                    nc.tensor.matmul(dps[:, 0:512], lhsT, rhs,
                                     start=True, stop=True)

            # ---- quantize (greedy engine balance) -------------------------
            q_rate_b = dict(q_rate)
            q_load_b = dict(q_load)
            if pool_quant == "burst":
                q_rate_b["p"] = 1.48
                q_load_b["p"] = 0.0

            def quant(dst_ap, src_ap, scale_ap, elems, burst=False):
                rates = q_rate_b if burst else q_rate
                loads = q_load_b if burst else q_load
                e = min(loads,
                        key=lambda k: loads[k] + elems * rates[k])
                loads[e] += elems * rates[e] + q_fix[e]
                if e == "v":
                    nc.vector.tensor_scalar(dst_ap, src_ap, scale_ap, None,
                                            op0=mybir.AluOpType.mult)
                elif e == "a":
                    nc.scalar.activation(dst_ap, src_ap,
                                         mybir.ActivationFunctionType.Copy,
                                         scale=scale_ap)
                else:
                    nc.gpsimd.tensor_scalar(dst_ap, src_ap, scale_ap, None,
                                            op0=mybir.AluOpType.mult)

            w8 = [w8pool.tile([128, 2 * NSH], F8, tag="w8", name=f"w8_{kb}")
                  for kb in range(KB)]
            x8 = {}
            for s in range(NSTRIPES):
                for kb in range(KB):
                    x8[(s, kb)] = x8pool.tile([128, 2 * SWM], F8, tag="x8",
                                              name=f"x8_{s}_{kb}")

            def w_src(kb):
                t = kb // 4            # wstg tile (KW k each, 8 chunks)
                c = 2 * kb - 8 * t
                return wstg[t][:, c:c + 2, :].rearrange("p a b -> p (a b)")

            def x_src(s, kb):
                t = kb // 4            # xcov/xstg tile (KW k, 8 chunks)
                c = 2 * kb - 8 * t
                return xstg[(s, t)][:, c:c + 2, :].rearrange(
                    "p a b -> p (a b)")

            # first burst: interleave w8 and x8 stripe-0 in kb order so the
            # PE can accumulate (w8[kb], x8[0,kb]) pairs as they appear
            for kb in range(KB):
                quant(w8[kb][:], w_src(kb), sw, 2 * NSH, burst=True)
                quant(x8[(0, kb)][:], x_src(0, kb), sx, 2 * SWM, burst=True)
            q_load["v"] += q_load_b["v"]
            q_load["a"] += q_load_b["a"]
            if preload:
                # deq+bias land on DVE (and out dispatch on its engine)
                # during the stripe phase; bias the remaining quant splits
                q_load["v"] += 29500.0 if deq == "dve" else 19000.0
                if out_eng == "act":
                    q_load["a"] += 10000.0
                if pool_quant and out_eng == "pool":
                    q_load["p"] += 16000.0

            # ---- matmul sweep ---------------------------------------------
            SPC = MCH // NSTRIPES
            for mc in range(MCH):
                s = mc // SPC
                lm = (mc % SPC) * 128
                sq = (mc + qlead) // SPC   # stripe whose quants to issue now
                if (mc + qlead) % SPC == 0 and 0 < sq < NSTRIPES:
                    for kb in range(KB):
                        quant(x8[(sq, kb)][:], x_src(sq, kb), sx, 2 * SWM)
                ps = ppool.tile([128, NSH], F32, tag="ps")
                if double_row:
                    for kb in range(KB):
                        lhsT = x8[(s, kb)].rearrange(
                            "p (i m) -> p i m", i=2)[:, :, lm:lm + 128]
                        rhs = w8[kb].rearrange("p (i n) -> p i n", i=2)
                        # accumulation group must stay within one 2KB PSUM
                        # bank (512 fp32): run the two n-halves separately
                        for nh in range(2):
                            nc.tensor.matmul(
                                ps[:, nh * (NSH // 2):(nh + 1) * (NSH // 2)],
                                lhsT, rhs[:, :, nh * (NSH // 2):
                                          (nh + 1) * (NSH // 2)],
                                start=(kb == 0), stop=(kb == KB - 1),
                                perf_mode=mybir.MatmulPerfMode.DoubleRow)
                else:
                    for kb in range(KB):
                        for i in range(2):
                            lhsT = x8[(s, kb)][:, i * SWM + lm:
                                               i * SWM + lm + 128]
                            rhs = w8[kb][:, i * NSH:(i + 1) * NSH]
                            nc.tensor.matmul(
                                ps[:], lhsT, rhs,
                                start=(kb == 0 and i == 0),
                                stop=(kb == KB - 1 and i == 1))
                ot = opool.tile([128, NSH], F16, tag="ot")
                if deq == "split":
                    nc.vector.tensor_scalar(ot[:, 0:NSH // 2],
                                            ps[:, 0:NSH // 2], r2[:], None,
                                            op0=mybir.AluOpType.mult)
                    nc.scalar.activation(ot[:, NSH // 2:], ps[:, NSH // 2:],
                                         mybir.ActivationFunctionType.Copy,
                                         scale=r2[:])
                elif deq == "dve" or (deq == "alt" and mc % 2 == 1):
                    nc.vector.tensor_scalar(ot[:], ps[:], r2[:], None,
                                            op0=mybir.AluOpType.mult)
                else:
                    nc.scalar.activation(ot[:], ps[:],
                                         mybir.ActivationFunctionType.Copy,
                                         scale=r2[:])
                nc.vector.tensor_tensor(ot[:], ot[:], bias_b[:],
                                        op=mybir.AluOpType.add)
                if out_eng == "act":
                    nc.scalar.dma_start(out[mc * 128:(mc + 1) * 128, :],
                                        ot[:])
                elif out_eng == "pool":
                    nc.gpsimd.dma_start(out[mc * 128:(mc + 1) * 128, :],
                                        ot[:])
                else:
                    nc.sync.dma_start(out[mc * 128:(mc + 1) * 128, :], ot[:])

    nc.compile()
    return nc


_CACHE = {}


def _get_kernel(M=4096, K=4096, NSH=None, SW=None, double_row=None):
    """NSH/SW args accepted for compatibility; config is fixed internally."""
    key = (M, K)
    if key not in _CACHE:
        _CACHE[key] = build_kernel(M, K, NSH=K // CGRP,
                                   double_row=DOUBLE_ROW)
    return _CACHE[key]


def kernel(x, weight, bias):
    M, K = x.shape
    N = weight.shape[0]
    nc = _get_kernel(M, K)
    MH, NSH = M // RGRP, N // CGRP
    SH = MH // CGRP           # x m-roll unit (x coverage distinctness)
    NR = NSH // RGRP          # w n-roll unit (w coverage distinctness)

    x = np.asarray(x)
    weight = np.asarray(weight)
    bias = np.asarray(bias)
    in_maps = []
    for core in range(NCORES):
        r, c = divmod(core, CGRP)
        xh = np.roll(x[r * MH:(r + 1) * MH], -SH * c, axis=0)
        wq = np.roll(weight[c * NSH:(c + 1) * NSH], -NR * r, axis=0)
        bq = np.roll(bias[c * NSH:(c + 1) * NSH], -NR * r)
        in_maps.append({
            "x": np.ascontiguousarray(xh),
            "w": np.ascontiguousarray(wq),
            "bias": np.ascontiguousarray(bq.reshape(1, NSH)),
        })
    # The axon terminal occasionally reports a stale NRT_EXEC_UNIT error from
    # a previous session on first use; a retry lands on a recovered device.
    last_err = None
    for _ in range(3):
        try:
            res = run_bass_kernel_spmd(nc, in_maps,
                                       core_ids=list(range(NCORES)))
            break
        except Exception as e:  # noqa: BLE001
            last_err = e
            time.sleep(2.0)
    else:
        raise last_err
    full = np.empty((M, N), dtype=np.float16)
    for core in range(NCORES):
        r, c = divmod(core, CGRP)
        o = np.asarray(res.results[core]["out"])
        o = np.roll(o, (SH * c, NR * r), axis=(0, 1))
        full[r * MH:(r + 1) * MH, c * NSH:(c + 1) * NSH] = o
    return full


# revision 42
# speedup vs baseline: 1.0115x; 1.0011x over previous
"""FP8 dynamic-quantized linear (nn_FP8Linear) on 8 Trainium2 NeuronCores.

out = fp16((x_fp8 @ w_fp8.T) / (sx*sw)) + bias, with per-tensor dynamic
fp8-e4m3 quantization of x and weight (scale = FP8_MAX / amax).

Sharding: 2x4 tensor-parallel grid. x rows split in 2 halves (replicated
across the 4 cores of a row group); weight/bias split in 4 column slabs
(replicated across the 2 cores of a column group). Each core computes a
[M/2, N/4] output slab; the host stitches the 8 slabs (no output
collective needed). This cuts per-core fp16 loads to 24MB vs 36MB for
out_features-only sharding.

Global per-tensor amaxes (must match the reference exactly) come from a
"coverage" scheme: each core's FIRST-loaded 8MB -- a distinct quarter of
its x half (m-stripe 0 after a host-side np.roll of the rows) and a
distinct n-half of its w slab (after a host-side n-roll) -- is
abs-max-reduced as it lands in SBUF, split between the DVE and GpSimd
engines so the reduction keeps pace with the DMA. Partials land in
columns of shared accumulators (one final reduce, no combine tree). The
8 cores' partial pairs are exchanged with one tiny AllGather (15us
modeled vs 28us for AllReduce) plus a local max; the union of the 8
coverage sets is exactly x and w, so the scales are the exact global
ones and quantization matches the reference bit-for-bit (modulo the
power-of-2 trick below). The rolls also let every core run the SAME
SPMD program; the host un-rolls the output slab.

The Tile scheduler serializes DmaTranspose against collectives (they
share the DMA/XBAR path), so w is loaded in NATURAL layout (plain DMA
overlaps the collective) and transposed to k-major on the otherwise-
idle PE (matmul-transpose against an identity, fp16 through PSUM is
exact), with psum->SBUF assembly copies on DVE/Act. x coverage is
DMA-transposed before the collective; the x remainder is DMA-transposed
after the scale readback (explicit dep) so it cannot delay the
collective, and output writes are dispatched from the Pool engine so
they never head-of-line-block the SP transpose stream.

Matmuls are fp8 DoubleRow (2x PE rate, 256-deep contraction per pass);
each accumulation group is split into 512-column halves because a
matmul accumulation group must stay inside one 2KB PSUM bank (the
walrus codegen rejects wider groups). Discarded fp16 matmuls bridge the
PE p-state through the amax/collective window.

TRN fp8e4 (float8_e4m3) has max +-240 vs OCP e4m3fn's +-448, so the
device uses scale 224/amax == ref_scale/2: fp8 grids are self-similar
under powers of two, so device fp8 values are exactly half the
reference's, and the dequant multipliers absorb the factor of 4.

Modeled (TimelineSim) exec time: 147729 ns vs 279277 ns for the
previous out_features-sharded kernel (1.89x).
"""

import time

import numpy as np

import concourse.bacc as bacc
import concourse.bass as bass
import concourse.bass_isa as bass_isa
import concourse.mybir as mybir
import concourse.tile as tile
from concourse import masks
from concourse.bass import _add_dep_helper
from concourse.bass_utils import run_bass_kernel_spmd

F16 = mybir.dt.float16
F32 = mybir.dt.float32
F8 = mybir.dt.float8e4

NCORES = 8
RGRP, CGRP = 2, 4       # row groups (x halves) x col groups (w slabs)
EPS = 1e-12
# device-side quantization scale numerator: ref uses 448 (e4m3fn max); we use
# 224 so quantized values stay within TRN e4m3's +-240 normal range.
DEV_FP8_MAX = 224.0
DOUBLE_ROW = True
POOL_QUANT = False
WARMUP = 30


def build_kernel(M=4096, K=4096, NSH=1024, double_row=True,
                 pool_quant=POOL_QUANT, warmup=WARMUP, out_eng="pool",
                 deq="dve", cp_act=False, preload=True, qlead=1):
    """Build + compile the per-core bass program.

    Per-core shapes: x [M/2, K], w [NSH, K], out [M/2, NSH] with NSH=N/4.
    double_row: fp8 DoubleRow matmuls (2x PE throughput, ~1e-4 rel noise).
    warmup: number of discarded fp16 matmuls (gated on the last w load)
    bridging the PE p-state between the w transposes and the fp8 burst.
    pool_quant: also use the gpsimd (Pool) engine for fp16->fp8 quantize.
    """
    MH = M // RGRP            # 2048 token rows per core
    KB = K // 256             # 16 k-blocks (DoubleRow contracts 256/pass)
    NSTRIPES = 4
    SWM = MH // NSTRIPES      # 512-row m-stripes
    MCH = MH // 128           # 16 m-chunks per core
    KW = K // 4               # transfer k-width (1024)
    KCH = K // 128            # 32 k-chunks
    WNT = NSH // 128          # 8 natural w tiles
    assert MH % NSTRIPES == 0 and K % 256 == 0

    nc = bacc.Bacc("TRN2", target_bir_lowering=False, debug=False,
                   num_devices=NCORES)
    x = nc.dram_tensor("x", [MH, K], F16, kind="ExternalInput").ap()
    w = nc.dram_tensor("w", [NSH, K], F16, kind="ExternalInput").ap()
    bias = nc.dram_tensor("bias", [1, NSH], F16, kind="ExternalInput").ap()
    out = nc.dram_tensor("out", [MH, NSH], F16, kind="ExternalOutput").ap()

    # greedy engine balancers (ns/elem/partition + fixed overhead),
    # calibrated against observed TimelineSim slice durations
    cp_rate = {"v": 2.2 if cp_act else 0.72, "a": 1.0}  # psum->SBUF copies
    cp_load = {k: 0.0 for k in cp_rate}
    q_rate = {"v": 0.52, "a": 0.92}               # fp16->fp8 quantize
    if pool_quant:
        q_rate["p"] = 1.48
    q_fix = {"v": 60.0, "a": 150.0, "p": 150.0}
    # reserve DVE for dequant+bias, Act for out-DMA dispatch, Pool for smalls
    q_load = {"v": 0.0, "a": 0.0}
    if pool_quant:
        q_load["p"] = 0.0

    DVE_SHARE = 0.57          # coverage amax: DVE share vs gpsimd

    with tile.TileContext(nc) as tc:
        with (
            tc.tile_pool(name="const", bufs=1) as cpool,
            tc.tile_pool(name="redu", bufs=16) as rpool,
            tc.tile_pool(name="nat", bufs=6) as natpool,
            tc.tile_pool(name="wstg", bufs=4) as wspool,
            tc.tile_pool(name="xstg", bufs=6) as xspool,
            tc.tile_pool(name="w8", bufs=KB) as w8pool,
            tc.tile_pool(name="x8", bufs=KB + 2) as x8pool,
            tc.tile_pool(name="psum", bufs=3, space="PSUM") as ppool,
            tc.tile_pool(name="tp", bufs=2, space="PSUM") as tppool,
            tc.tile_pool(name="ot", bufs=4) as opool,
            tc.tile_pool(name="dram", bufs=2, space="DRAM") as dpool,
        ):
            # ---- constants ------------------------------------------------
            bias_row = cpool.tile([1, NSH], F16, tag="bias_row")
            nc.gpsimd.dma_start(bias_row[:], bias[:])
            bias_b = cpool.tile([128, NSH], F16, tag="bias_b")
            nc.gpsimd.partition_broadcast(bias_b[:], bias_row[:])
            ident = cpool.tile([128, 128], F16, tag="ident")
            masks.make_identity(nc, ident[:])

            # partial amaxes land in columns of shared accumulators; one
            # final reduce replaces a pairwise combine tree
            dax = rpool.tile([128, 8], F32, tag="dax")
            daw = rpool.tile([128, 8], F32, tag="daw")
            pax = rpool.tile([1, 8], F32, tag="pax")
            paw = rpool.tile([1, 8], F32, tag="paw")
            nc.gpsimd.memset(dax[:], 0.0)
            nc.gpsimd.memset(daw[:], 0.0)
            nc.gpsimd.memset(pax[:], 0.0)
            nc.gpsimd.memset(paw[:], 0.0)
            n_d = {"x": 0, "w": 0}

            def amax_of(flat_ap, free, tag):
                h = int(free * DVE_SHARE) & ~63
                da = dax if tag == "x" else daw
                pa = pax if tag == "x" else paw
                i = n_d[tag]
                n_d[tag] += 1
                nc.vector.tensor_reduce(
                    da[:, i:i + 1], flat_ap[:, 0:h],
                    axis=mybir.AxisListType.X,
                    op=mybir.AluOpType.max, apply_absolute_value=True)
                nc.gpsimd.tensor_reduce(
                    pa[:, i:i + 1], flat_ap[:, h:free],
                    axis=mybir.AxisListType.XYZWC,
                    op=mybir.AluOpType.max, apply_absolute_value=True)

            # ---- w natural loads + PE transposes into k-major wstg --------
            # Half-tiles [128 n, K/2] keep the load->transpose->reuse chain
            # fine-grained so DMA never waits on the PE. After the host
            # n-roll, tiles nt<4 are this core's distinct amax coverage.
            def cp(dst_ap, src_ap, elems):
                e = min(cp_load,
                        key=lambda k: cp_load[k] + elems * cp_rate[k])
                cp_load[e] += elems * cp_rate[e] + 250.0
                if e == "v":
                    nc.vector.tensor_copy(dst_ap, src_ap)
                else:
                    nc.scalar.activation(dst_ap, src_ap,
                                         mybir.ActivationFunctionType.Copy)

            wstg = [wspool.tile([128, 8, NSH], F16, tag="wstg",
                                name=f"wstg_{g}") for g in range(4)]
            wnat = {}

            def load_wnat(nt, h):
                nat = natpool.tile([128, K // 2], F16, tag="nat",
                                   name=f"wnat_{nt}_{h}")
                nc.sync.dma_start(
                    nat[:], w[nt * 128:(nt + 1) * 128,
                              h * (K // 2):(h + 1) * (K // 2)])
                wnat[(nt, h)] = nat
                if nt < 4:
                    amax_of(nat[:], K // 2, "w")
                for g in range(2):
                    pst = tppool.tile([128, 8, 128], F16, tag="tp",
                                      name=f"tp_{nt}_{h}_{g}")
                    for j in range(8):
                        c = 8 * g + j
                        nc.tensor.transpose(
                            pst[:, j, :], nat[:, c * 128:(c + 1) * 128],
                            ident[:])
                    cp(wstg[2 * h + g][:, 0:8, nt * 128:(nt + 1) * 128],
                       pst[:], 8 * 128)

            for nt in range(4):
                for h in range(2):
                    load_wnat(nt, h)

            # ---- x stripe-0 coverage: natural half-tiles + PE transpose ---
            # (plain DMA keeps the collective window free of DmaTranspose)
            xstg = {}
            for t in range(4):
                xstg[(0, t)] = xspool.tile([128, KW // 128, SWM], F16,
                                           tag="xstg", name=f"xcov_{t}")
            for mt in range(SWM // 128):
                for h in range(2):
                    nat = natpool.tile([128, K // 2], F16, tag="nat",
                                       name=f"xnat_{mt}_{h}")
                    nc.sync.dma_start(
                        nat[:], x[mt * 128:(mt + 1) * 128,
                                  h * (K // 2):(h + 1) * (K // 2)])
                    amax_of(nat[:], K // 2, "x")
                    for g in range(2):
                        pst = tppool.tile([128, 8, 128], F16, tag="tp",
                                          name=f"xtp_{mt}_{h}_{g}")
                        for j in range(8):
                            c = 8 * g + j
                            nc.tensor.transpose(
                                pst[:, j, :], nat[:, c * 128:(c + 1) * 128],
                                ident[:])
                        cp(xstg[(0, 2 * h + g)][:, 0:8,
                                                mt * 128:(mt + 1) * 128],
                           pst[:], 8 * 128)

            # ---- w rest (overlaps the collective: plain DMA) --------------
            for nt in range(4, WNT):
                for h in range(2):
                    load_wnat(nt, h)

            # ---- AllGather(concat) global amaxes --------------------------
            _hp = tc.high_priority()
            _hp.__enter__()
            amax2 = rpool.tile([128, 2], F32, tag="amax2")
            nc.vector.tensor_reduce(amax2[:, 0:1], dax[:],
                                    axis=mybir.AxisListType.X,
                                    op=mybir.AluOpType.max)
            nc.vector.tensor_reduce(amax2[:, 1:2], daw[:],
                                    axis=mybir.AxisListType.X,
                                    op=mybir.AluOpType.max)
            amax2r = rpool.tile([128, 2], F32, tag="amax2r")
            nc.gpsimd.partition_all_reduce(
                amax2r[:], amax2[:], channels=128,
                reduce_op=bass_isa.ReduceOp.max)
            p2 = rpool.tile([1, 2], F32, tag="p2")
            nc.vector.tensor_reduce(p2[:, 0:1], pax[:],
                                    axis=mybir.AxisListType.X,
                                    op=mybir.AluOpType.max)
            nc.vector.tensor_reduce(p2[:, 1:2], paw[:],
                                    axis=mybir.AxisListType.X,
                                    op=mybir.AluOpType.max)
            bin2 = rpool.tile([1, 2], F32, tag="bin2")
            nc.vector.tensor_tensor(bin2[:], amax2r[0:1, :], p2[:],
                                    op=mybir.AluOpType.max)

            bin_ = dpool.tile([1, 2], F32, name="bin_")
            bout = dpool.tile([1, 2 * NCORES], F32, name="bout")
            nc.gpsimd.dma_start(bin_[:], bin2[:])
            cc = nc.gpsimd.collective_compute(
                "AllGather", mybir.AluOpType.bypass,
                replica_groups=[list(range(NCORES))],
                ins=[bin_.opt()], outs=[bout.opt()])
            g16 = rpool.tile([1, 2 * NCORES], F32, tag="g16")
            g16_read = nc.gpsimd.dma_start(g16[:], bout[:])
            # gathered layout: [c0x, c0w, c1x, c1w, ...] -> max over cores
            gm = rpool.tile([1, 2], F32, tag="gm")
            nc.vector.tensor_reduce(
                gm[:], g16[:].rearrange("a (g t) -> a t g", t=2),
                axis=mybir.AxisListType.X, op=mybir.AluOpType.max)
            nc.vector.tensor_scalar_max(gm[:], gm[:], EPS)
            gb = rpool.tile([128, 2], F32, tag="gb")
            nc.gpsimd.partition_broadcast(gb[:], gm[:])

            # scales: s = 224/amax (quant), r = 1/s (dequant), r2 = rx*rw
            u2 = rpool.tile([128, 2], F32, tag="u2")
            nc.vector.reciprocal(u2[:], gb[:])
            s2 = rpool.tile([128, 2], F32, tag="s2")
            nc.vector.tensor_scalar_mul(s2[:], u2[:], DEV_FP8_MAX)
            inv2 = rpool.tile([128, 2], F32, tag="inv2")
            nc.vector.reciprocal(inv2[:], s2[:])
            r2 = rpool.tile([128, 1], F32, tag="r2")
            nc.vector.tensor_tensor(r2[:], inv2[:, 0:1], inv2[:, 1:2],
                                    op=mybir.AluOpType.mult)
            sx, sw = s2[:, 0:1], s2[:, 1:2]
            _hp.__exit__(None, None, None)

            # ---- stripe-1 k-half 0: natural loads + PE transpose ----------
            # (plain DMA fills the collective-window DMA idle; only tiles
            # (1,0)/(1,1) have free staging bufs this early)
            for t in range(2):
                xstg[(1, t)] = xspool.tile([128, KW // 128, SWM], F16,
                                           tag="xstg", name=f"xstg_1_{t}")
            for mt in range(SWM // 128):
                nat = natpool.tile([128, K // 2], F16, tag="nat",
                                   name=f"x1nat_{mt}")
                nc.sync.dma_start(
                    nat[:], x[SWM + mt * 128:SWM + (mt + 1) * 128,
                              0:K // 2])
                for g in range(2):
                    pst = tppool.tile([128, 8, 128], F16, tag="tp",
                                      name=f"x1tp_{mt}_{g}")
                    for j in range(8):
                        c = 8 * g + j
                        nc.tensor.transpose(
                            pst[:, j, :], nat[:, c * 128:(c + 1) * 128],
                            ident[:])
                    cp(xstg[(1, g)][:, 0:8, mt * 128:(mt + 1) * 128],
                       pst[:], 8 * 128)

            # ---- x rest: transposed loads AFTER the readback --------------
            # (DmaTranspose serializes against the collective; gating these
            # on the readback keeps the collective + scales path clean.)
            for s in range(1, NSTRIPES):
                for q in range(4):
                    if s == 1 and q < 2:
                        continue
                    stg = xspool.tile([128, KW // 128, SWM], F16,
                                      tag="xstg", name=f"xstg_{s}_{q}")
                    d = nc.sync.dma_start(
                        stg[:], x[s * SWM:(s + 1) * SWM,
                                  q * KW:(q + 1) * KW],
                        transpose=True)
                    _add_dep_helper(d.ins, cc.ins, sync=True,
                                    reason="hold transposes off collective")
                    xstg[(s, q)] = stg

            # ---- PE p-state bridge: discarded fp16 matmuls ----------------
            if warmup:
                dps = ppool.tile([128, NSH], F32, tag="ps", name="dps")
                rhs = wnat[(WNT - 1, 1)][:, 0:512]
                lhsT = wnat[(WNT - 1, 1)][:, 512:640]
                for _ in range(warmup):
                    nc.tensor.matmul(dps[:, 0:512], lhsT, rhs,
                                     start=True, stop=True)

            # ---- quantize (greedy engine balance) -------------------------
            q_rate_b = dict(q_rate)
            q_load_b = dict(q_load)
            if pool_quant == "burst":
                q_rate_b["p"] = 1.48
                q_load_b["p"] = 0.0

            def quant(dst_ap, src_ap, scale_ap, elems, burst=False):
                rates = q_rate_b if burst else q_rate
                loads = q_load_b if burst else q_load
                e = min(loads,
                        key=lambda k: loads[k] + elems * rates[k])
                loads[e] += elems * rates[e] + q_fix[e]
                if e == "v":
                    nc.vector.tensor_scalar(dst_ap, src_ap, scale_ap, None,
                                            op0=mybir.AluOpType.mult)
                elif e == "a":
                    nc.scalar.activation(dst_ap, src_ap,
                                         mybir.ActivationFunctionType.Copy,
                                         scale=scale_ap)
                else:
                    nc.gpsimd.tensor_scalar(dst_ap, src_ap, scale_ap, None,
                                            op0=mybir.AluOpType.mult)

            w8 = [w8pool.tile([128, 2 * NSH], F8, tag="w8", name=f"w8_{kb}")
                  for kb in range(KB)]
            x8 = {}
            for s in range(NSTRIPES):
                for kb in range(KB):
                    x8[(s, kb)] = x8pool.tile([128, 2 * SWM], F8, tag="x8",
                                              name=f"x8_{s}_{kb}")

            def w_src(kb):
                t = kb // 4            # wstg tile (KW k each, 8 chunks)
                c = 2 * kb - 8 * t
                return wstg[t][:, c:c + 2, :].rearrange("p a b -> p (a b)")

            def x_src(s, kb):
                t = kb // 4            # xcov/xstg tile (KW k, 8 chunks)
                c = 2 * kb - 8 * t
                return xstg[(s, t)][:, c:c + 2, :].rearrange(
                    "p a b -> p (a b)")

            # first burst: interleave w8 and x8 stripe-0 in kb order so the
            # PE can accumulate (w8[kb], x8[0,kb]) pairs as they appear
            for kb in range(KB):
                quant(w8[kb][:], w_src(kb), sw, 2 * NSH, burst=True)
                quant(x8[(0, kb)][:], x_src(0, kb), sx, 2 * SWM, burst=True)
            q_load["v"] += q_load_b["v"]
            q_load["a"] += q_load_b["a"]
            if preload:
                # deq+bias land on DVE (and out dispatch on its engine)
                # during the stripe phase; bias the remaining quant splits
                q_load["v"] += 29500.0 if deq == "dve" else 19000.0
                if out_eng == "act":
                    q_load["a"] += 10000.0
                if pool_quant and out_eng == "pool":
                    q_load["p"] += 16000.0

            # ---- matmul sweep ---------------------------------------------
            SPC = MCH // NSTRIPES
            for mc in range(MCH):
                s = mc // SPC
                lm = (mc % SPC) * 128
                sq = (mc + qlead) // SPC   # stripe whose quants to issue now
                if (mc + qlead) % SPC == 0 and 0 < sq < NSTRIPES:
                    for kb in range(KB):
                        quant(x8[(sq, kb)][:], x_src(sq, kb), sx, 2 * SWM)
                ps = ppool.tile([128, NSH], F32, tag="ps")
                if double_row:
                    for kb in range(KB):
                        lhsT = x8[(s, kb)].rearrange(
                            "p (i m) -> p i m", i=2)[:, :, lm:lm + 128]
                        rhs = w8[kb].rearrange("p (i n) -> p i n", i=2)
                        # accumulation group must stay within one 2KB PSUM
                        # bank (512 fp32): run the two n-halves separately
                        for nh in range(2):
                            nc.tensor.matmul(
                                ps[:, nh * (NSH // 2):(nh + 1) * (NSH // 2)],
                                lhsT, rhs[:, :, nh * (NSH // 2):
                                          (nh + 1) * (NSH // 2)],
                                start=(kb == 0), stop=(kb == KB - 1),
                                perf_mode=mybir.MatmulPerfMode.DoubleRow)
                else:
                    for kb in range(KB):
                        for i in range(2):
                            lhsT = x8[(s, kb)][:, i * SWM + lm:
                                               i * SWM + lm + 128]
                            rhs = w8[kb][:, i * NSH:(i + 1) * NSH]
                            nc.tensor.matmul(
                                ps[:], lhsT, rhs,
                                start=(kb == 0 and i == 0),
                                stop=(kb == KB - 1 and i == 1))
                ot = opool.tile([128, NSH], F16, tag="ot")
                if deq == "split":
                    nc.vector.tensor_scalar(ot[:, 0:NSH // 2],
                                            ps[:, 0:NSH // 2], r2[:], None,
                                            op0=mybir.AluOpType.mult)
                    nc.scalar.activation(ot[:, NSH // 2:], ps[:, NSH // 2:],
                                         mybir.ActivationFunctionType.Copy,
                                         scale=r2[:])
                elif deq == "dve" or (deq == "alt" and mc % 2 == 1):
                    nc.vector.tensor_scalar(ot[:], ps[:], r2[:], None,
                                            op0=mybir.AluOpType.mult)
                else:
                    nc.scalar.activation(ot[:], ps[:],
                                         mybir.ActivationFunctionType.Copy,
                                         scale=r2[:])
                nc.vector.tensor_tensor(ot[:], ot[:], bias_b[:],
                                        op=mybir.AluOpType.add)
                if out_eng == "act":
                    nc.scalar.dma_start(out[mc * 128:(mc + 1) * 128, :],
                                        ot[:])
                elif out_eng == "pool":
                    nc.gpsimd.dma_start(out[mc * 128:(mc + 1) * 128, :],
                                        ot[:])
                else:
                    nc.sync.dma_start(out[mc * 128:(mc + 1) * 128, :], ot[:])

    nc.compile()
    return nc


_CACHE = {}


def _get_kernel(M=4096, K=4096, NSH=None, SW=None, double_row=None):
    """NSH/SW args accepted for compatibility; config is fixed internally."""
    key = (M, K)
    if key not in _CACHE:
        _CACHE[key] = build_kernel(M, K, NSH=K // CGRP,
                                   double_row=DOUBLE_ROW)
    return _CACHE[key]


def kernel(x, weight, bias):
    M, K = x.shape
    N = weight.shape[0]
    nc = _get_kernel(M, K)
    MH, NSH = M // RGRP, N // CGRP
    SH = MH // CGRP           # x m-roll unit (x coverage distinctness)
    NR = NSH // RGRP          # w n-roll unit (w coverage distinctness)

    x = np.asarray(x)
    weight = np.asarray(weight)
    bias = np.asarray(bias)
    in_maps = []
    for core in range(NCORES):
        r, c = divmod(core, CGRP)
        xh = np.roll(x[r * MH:(r + 1) * MH], -SH * c, axis=0)
        wq = np.roll(weight[c * NSH:(c + 1) * NSH], -NR * r, axis=0)
        bq = np.roll(bias[c * NSH:(c + 1) * NSH], -NR * r)
        in_maps.append({
            "x": np.ascontiguousarray(xh),
            "w": np.ascontiguousarray(wq),
            "bias": np.ascontiguousarray(bq.reshape(1, NSH)),
        })
    # The axon terminal occasionally reports a stale NRT_EXEC_UNIT error from
    # a previous session on first use; a retry lands on a recovered device.
    last_err = None
    for _ in range(3):
        try:
            res = run_bass_kernel_spmd(nc, in_maps,
                                       core_ids=list(range(NCORES)))
            break
        except Exception as e:  # noqa: BLE001
            last_err = e
            time.sleep(2.0)
    else:
        raise last_err
    full = np.empty((M, N), dtype=np.float16)
    for core in range(NCORES):
        r, c = divmod(core, CGRP)
        o = np.asarray(res.results[core]["out"])
        o = np.roll(o, (SH * c, NR * r), axis=(0, 1))
        full[r * MH:(r + 1) * MH, c * NSH:(c + 1) * NSH] = o
    return full


# revision 44
# speedup vs baseline: 1.0236x; 1.0120x over previous
"""FP8 dynamic-quantized linear (nn_FP8Linear) on 8 Trainium2 NeuronCores.

out = fp16((x_fp8 @ w_fp8.T) / (sx*sw)) + bias, with per-tensor dynamic
fp8-e4m3 quantization of x and weight (scale = FP8_MAX / amax).

Sharding: 2x4 tensor-parallel grid. x rows split in 2 halves (replicated
across the 4 cores of a row group); weight/bias split in 4 column slabs
(replicated across the 2 cores of a column group). Each core computes a
[M/2, N/4] output slab; the host stitches the 8 slabs (no output
collective needed). This cuts per-core fp16 loads to 24MB vs 36MB for
out_features-only sharding.

Global per-tensor amaxes (must match the reference exactly) come from a
"coverage" scheme: each core's FIRST-loaded 8MB -- a distinct quarter of
its x half (m-stripe 0 after a host-side np.roll of the rows) and a
distinct n-half of its w slab (after a host-side n-roll) -- is
abs-max-reduced as it lands in SBUF, split between the DVE and GpSimd
engines so the reduction keeps pace with the DMA. Partials land in
columns of shared accumulators (one final reduce, no combine tree). The
8 cores' partial pairs are exchanged with one tiny AllGather (15us
modeled vs 28us for AllReduce) plus a local max; the union of the 8
coverage sets is exactly x and w, so the scales are the exact global
ones and quantization matches the reference bit-for-bit (modulo the
power-of-2 trick below). The rolls also let every core run the SAME
SPMD program; the host un-rolls the output slab.

The Tile scheduler serializes DmaTranspose against collectives (they
share the DMA/XBAR path), so w is loaded in NATURAL layout (plain DMA
overlaps the collective) and transposed to k-major on the otherwise-
idle PE (matmul-transpose against an identity, fp16 through PSUM is
exact), with psum->SBUF assembly copies on DVE/Act. x coverage is
DMA-transposed before the collective; the x remainder is DMA-transposed
after the scale readback (explicit dep) so it cannot delay the
collective, and output writes are dispatched from the Pool engine so
they never head-of-line-block the SP transpose stream.

Matmuls are fp8 DoubleRow (2x PE rate, 256-deep contraction per pass);
each accumulation group is split into 512-column halves because a
matmul accumulation group must stay inside one 2KB PSUM bank (the
walrus codegen rejects wider groups). Discarded fp16 matmuls bridge the
PE p-state through the amax/collective window.

TRN fp8e4 (float8_e4m3) has max +-240 vs OCP e4m3fn's +-448, so the
device uses scale 224/amax == ref_scale/2: fp8 grids are self-similar
under powers of two, so device fp8 values are exactly half the
reference's, and the dequant multipliers absorb the factor of 4.

Modeled (TimelineSim) exec time: 145819 ns vs 279277 ns for the
previous out_features-sharded kernel (1.92x).
"""

import time

import numpy as np

import concourse.bacc as bacc
import concourse.bass as bass
import concourse.bass_isa as bass_isa
import concourse.mybir as mybir
import concourse.tile as tile
from concourse import masks
from concourse.bass import _add_dep_helper
from concourse.bass_utils import run_bass_kernel_spmd

F16 = mybir.dt.float16
F32 = mybir.dt.float32
F8 = mybir.dt.float8e4

NCORES = 8
RGRP, CGRP = 2, 4       # row groups (x halves) x col groups (w slabs)
EPS = 1e-12
# device-side quantization scale numerator: ref uses 448 (e4m3fn max); we use
# 224 so quantized values stay within TRN e4m3's +-240 normal range.
DEV_FP8_MAX = 224.0
DOUBLE_ROW = True
POOL_QUANT = False
WARMUP = 30


def build_kernel(M=4096, K=4096, NSH=1024, double_row=True,
                 pool_quant=POOL_QUANT, warmup=WARMUP, out_eng="pool",
                 deq="dve", cp_act=False, preload=True, qlead=1):
    """Build + compile the per-core bass program.

    Per-core shapes: x [M/2, K], w [NSH, K], out [M/2, NSH] with NSH=N/4.
    double_row: fp8 DoubleRow matmuls (2x PE throughput, ~1e-4 rel noise).
    warmup: number of discarded fp16 matmuls (gated on the last w load)
    bridging the PE p-state between the w transposes and the fp8 burst.
    pool_quant: also use the gpsimd (Pool) engine for fp16->fp8 quantize.
    """
    MH = M // RGRP            # 2048 token rows per core
    KB = K // 256             # 16 k-blocks (DoubleRow contracts 256/pass)
    NSTRIPES = 4
    SWM = MH // NSTRIPES      # 512-row m-stripes
    MCH = MH // 128           # 16 m-chunks per core
    KW = K // 4               # transfer k-width (1024)
    KCH = K // 128            # 32 k-chunks
    WNT = NSH // 128          # 8 natural w tiles
    assert MH % NSTRIPES == 0 and K % 256 == 0

    nc = bacc.Bacc("TRN2", target_bir_lowering=False, debug=False,
                   num_devices=NCORES)
    x = nc.dram_tensor("x", [MH, K], F16, kind="ExternalInput").ap()
    w = nc.dram_tensor("w", [NSH, K], F16, kind="ExternalInput").ap()
    bias = nc.dram_tensor("bias", [1, NSH], F16, kind="ExternalInput").ap()
    out = nc.dram_tensor("out", [MH, NSH], F16, kind="ExternalOutput").ap()

    # greedy engine balancers (ns/elem/partition + fixed overhead),
    # calibrated against observed TimelineSim slice durations
    cp_rate = {"v": 2.2 if cp_act else 0.72, "a": 1.0}  # psum->SBUF copies
    cp_load = {k: 0.0 for k in cp_rate}
    q_rate = {"v": 0.52, "a": 0.92}               # fp16->fp8 quantize
    if pool_quant:
        q_rate["p"] = 1.48
    q_fix = {"v": 60.0, "a": 150.0, "p": 150.0}
    # reserve DVE for dequant+bias, Act for out-DMA dispatch, Pool for smalls
    q_load = {"v": 0.0, "a": 0.0}
    if pool_quant:
        q_load["p"] = 0.0

    DVE_SHARE = 0.45          # coverage amax: DVE share vs gpsimd

    with tile.TileContext(nc) as tc:
        with (
            tc.tile_pool(name="const", bufs=1) as cpool,
            tc.tile_pool(name="redu", bufs=16) as rpool,
            tc.tile_pool(name="nat", bufs=6) as natpool,
            tc.tile_pool(name="wstg", bufs=4) as wspool,
            tc.tile_pool(name="xstg", bufs=6) as xspool,
            tc.tile_pool(name="w8", bufs=KB) as w8pool,
            tc.tile_pool(name="x8", bufs=KB + 2) as x8pool,
            tc.tile_pool(name="psum", bufs=3, space="PSUM") as ppool,
            tc.tile_pool(name="tp", bufs=2, space="PSUM") as tppool,
            tc.tile_pool(name="ot", bufs=4) as opool,
            tc.tile_pool(name="dram", bufs=2, space="DRAM") as dpool,
        ):
            # ---- constants ------------------------------------------------
            bias_row = cpool.tile([1, NSH], F16, tag="bias_row")
            nc.gpsimd.dma_start(bias_row[:], bias[:])
            bias_b = cpool.tile([128, NSH], F16, tag="bias_b")
            nc.gpsimd.partition_broadcast(bias_b[:], bias_row[:])
            ident = cpool.tile([128, 128], F16, tag="ident")
            masks.make_identity(nc, ident[:])

            # partial amaxes land in columns of shared accumulators; one
            # final reduce replaces a pairwise combine tree
            dax = rpool.tile([128, 8], F32, tag="dax")
            daw = rpool.tile([128, 8], F32, tag="daw")
            pax = rpool.tile([1, 8], F32, tag="pax")
            paw = rpool.tile([1, 8], F32, tag="paw")
            nc.gpsimd.memset(dax[:], 0.0)
            nc.gpsimd.memset(daw[:], 0.0)
            nc.gpsimd.memset(pax[:], 0.0)
            nc.gpsimd.memset(paw[:], 0.0)
            n_d = {"x": 0, "w": 0}

            def amax_of(flat_ap, free, tag):
                h = int(free * DVE_SHARE) & ~63
                da = dax if tag == "x" else daw
                pa = pax if tag == "x" else paw
                i = n_d[tag]
                n_d[tag] += 1
                nc.vector.tensor_reduce(
                    da[:, i:i + 1], flat_ap[:, 0:h],
                    axis=mybir.AxisListType.X,
                    op=mybir.AluOpType.max, apply_absolute_value=True)
                nc.gpsimd.tensor_reduce(
                    pa[:, i:i + 1], flat_ap[:, h:free],
                    axis=mybir.AxisListType.XYZWC,
                    op=mybir.AluOpType.max, apply_absolute_value=True)

            # ---- w natural loads + PE transposes into k-major wstg --------
            # Half-tiles [128 n, K/2] keep the load->transpose->reuse chain
            # fine-grained so DMA never waits on the PE. After the host
            # n-roll, tiles nt<4 are this core's distinct amax coverage.
            def cp(dst_ap, src_ap, elems):
                e = min(cp_load,
                        key=lambda k: cp_load[k] + elems * cp_rate[k])
                cp_load[e] += elems * cp_rate[e] + 250.0
                if e == "v":
                    nc.vector.tensor_copy(dst_ap, src_ap)
                else:
                    nc.scalar.activation(dst_ap, src_ap,
                                         mybir.ActivationFunctionType.Copy)

            wstg = [wspool.tile([128, 8, NSH], F16, tag="wstg",
                                name=f"wstg_{g}") for g in range(4)]
            wnat = {}

            def load_wnat(nt, h):
                nat = natpool.tile([128, K // 2], F16, tag="nat",
                                   name=f"wnat_{nt}_{h}")
                nc.sync.dma_start(
                    nat[:], w[nt * 128:(nt + 1) * 128,
                              h * (K // 2):(h + 1) * (K // 2)])
                wnat[(nt, h)] = nat
                if nt < 4:
                    amax_of(nat[:], K // 2, "w")
                for g in range(2):
                    pst = tppool.tile([128, 8, 128], F16, tag="tp",
                                      name=f"tp_{nt}_{h}_{g}")
                    for j in range(8):
                        c = 8 * g + j
                        nc.tensor.transpose(
                            pst[:, j, :], nat[:, c * 128:(c + 1) * 128],
                            ident[:])
                    cp(wstg[2 * h + g][:, 0:8, nt * 128:(nt + 1) * 128],
                       pst[:], 8 * 128)

            for nt in range(4):
                for h in range(2):
                    load_wnat(nt, h)

            # ---- x stripe-0 coverage: natural half-tiles + PE transpose ---
            # (plain DMA keeps the collective window free of DmaTranspose)
            xstg = {}
            for t in range(4):
                xstg[(0, t)] = xspool.tile([128, KW // 128, SWM], F16,
                                           tag="xstg", name=f"xcov_{t}")
            for mt in range(SWM // 128):
                for h in range(2):
                    nat = natpool.tile([128, K // 2], F16, tag="nat",
                                       name=f"xnat_{mt}_{h}")
                    nc.sync.dma_start(
                        nat[:], x[mt * 128:(mt + 1) * 128,
                                  h * (K // 2):(h + 1) * (K // 2)])
                    amax_of(nat[:], K // 2, "x")
                    for g in range(2):
                        pst = tppool.tile([128, 8, 128], F16, tag="tp",
                                          name=f"xtp_{mt}_{h}_{g}")
                        for j in range(8):
                            c = 8 * g + j
                            nc.tensor.transpose(
                                pst[:, j, :], nat[:, c * 128:(c + 1) * 128],
                                ident[:])
                        cp(xstg[(0, 2 * h + g)][:, 0:8,
                                                mt * 128:(mt + 1) * 128],
                           pst[:], 8 * 128)

            # ---- w rest (overlaps the collective: plain DMA) --------------
            for nt in range(4, WNT):
                for h in range(2):
                    load_wnat(nt, h)

            # ---- AllGather(concat) global amaxes --------------------------
            _hp = tc.high_priority()
            _hp.__enter__()
            amax2 = rpool.tile([128, 2], F32, tag="amax2")
            nc.vector.tensor_reduce(amax2[:, 0:1], dax[:],
                                    axis=mybir.AxisListType.X,
                                    op=mybir.AluOpType.max)
            nc.vector.tensor_reduce(amax2[:, 1:2], daw[:],
                                    axis=mybir.AxisListType.X,
                                    op=mybir.AluOpType.max)
            amax2r = rpool.tile([128, 2], F32, tag="amax2r")
            nc.gpsimd.partition_all_reduce(
                amax2r[:], amax2[:], channels=128,
                reduce_op=bass_isa.ReduceOp.max)
            p2 = rpool.tile([1, 2], F32, tag="p2")
            nc.vector.tensor_reduce(p2[:, 0:1], pax[:],
                                    axis=mybir.AxisListType.X,
                                    op=mybir.AluOpType.max)
            nc.vector.tensor_reduce(p2[:, 1:2], paw[:],
                                    axis=mybir.AxisListType.X,
                                    op=mybir.AluOpType.max)
            bin2 = rpool.tile([1, 2], F32, tag="bin2")
            nc.vector.tensor_tensor(bin2[:], amax2r[0:1, :], p2[:],
                                    op=mybir.AluOpType.max)

            bin_ = dpool.tile([1, 2], F32, name="bin_")
            bout = dpool.tile([1, 2 * NCORES], F32, name="bout")
            nc.gpsimd.dma_start(bin_[:], bin2[:])
            cc = nc.gpsimd.collective_compute(
                "AllGather", mybir.AluOpType.bypass,
                replica_groups=[list(range(NCORES))],
                ins=[bin_.opt()], outs=[bout.opt()])
            g16 = rpool.tile([1, 2 * NCORES], F32, tag="g16")
            g16_read = nc.gpsimd.dma_start(g16[:], bout[:])
            # gathered layout: [c0x, c0w, c1x, c1w, ...] -> max over cores
            gm = rpool.tile([1, 2], F32, tag="gm")
            nc.vector.tensor_reduce(
                gm[:], g16[:].rearrange("a (g t) -> a t g", t=2),
                axis=mybir.AxisListType.X, op=mybir.AluOpType.max)
            nc.vector.tensor_scalar_max(gm[:], gm[:], EPS)
            gb = rpool.tile([128, 2], F32, tag="gb")
            nc.gpsimd.partition_broadcast(gb[:], gm[:])

            # scales: s = 224/amax (quant), r = 1/s (dequant), r2 = rx*rw
            u2 = rpool.tile([128, 2], F32, tag="u2")
            nc.vector.reciprocal(u2[:], gb[:])
            s2 = rpool.tile([128, 2], F32, tag="s2")
            nc.vector.tensor_scalar_mul(s2[:], u2[:], DEV_FP8_MAX)
            inv2 = rpool.tile([128, 2], F32, tag="inv2")
            nc.vector.reciprocal(inv2[:], s2[:])
            r2 = rpool.tile([128, 1], F32, tag="r2")
            nc.vector.tensor_tensor(r2[:], inv2[:, 0:1], inv2[:, 1:2],
                                    op=mybir.AluOpType.mult)
            sx, sw = s2[:, 0:1], s2[:, 1:2]
            _hp.__exit__(None, None, None)

            # ---- stripe-1 k-half 0: natural loads + PE transpose ----------
            # (plain DMA fills the collective-window DMA idle; only tiles
            # (1,0)/(1,1) have free staging bufs this early)
            for t in range(2):
                xstg[(1, t)] = xspool.tile([128, KW // 128, SWM], F16,
                                           tag="xstg", name=f"xstg_1_{t}")
            for mt in range(SWM // 128):
                nat = natpool.tile([128, K // 2], F16, tag="nat",
                                   name=f"x1nat_{mt}")
                nc.sync.dma_start(
                    nat[:], x[SWM + mt * 128:SWM + (mt + 1) * 128,
                              0:K // 2])
                for g in range(2):
                    pst = tppool.tile([128, 8, 128], F16, tag="tp",
                                      name=f"x1tp_{mt}_{g}")
                    for j in range(8):
                        c = 8 * g + j
                        nc.tensor.transpose(
                            pst[:, j, :], nat[:, c * 128:(c + 1) * 128],
                            ident[:])
                    cp(xstg[(1, g)][:, 0:8, mt * 128:(mt + 1) * 128],
                       pst[:], 8 * 128)

            # ---- x rest: transposed loads AFTER the readback --------------
            # (DmaTranspose serializes against the collective; gating these
            # on the readback keeps the collective + scales path clean.)
            for s in range(1, NSTRIPES):
                for q in range(4):
                    if s == 1 and q < 2:
                        continue
                    stg = xspool.tile([128, KW // 128, SWM], F16,
                                      tag="xstg", name=f"xstg_{s}_{q}")
                    d = nc.sync.dma_start(
                        stg[:], x[s * SWM:(s + 1) * SWM,
                                  q * KW:(q + 1) * KW],
                        transpose=True)
                    _add_dep_helper(d.ins, cc.ins, sync=True,
                                    reason="hold transposes off collective")
                    xstg[(s, q)] = stg

            # ---- PE p-state bridge: discarded fp16 matmuls ----------------
            if warmup:
                dps = ppool.tile([128, NSH], F32, tag="ps", name="dps")
                rhs = wnat[(WNT - 1, 1)][:, 0:512]
                lhsT = wnat[(WNT - 1, 1)][:, 512:640]
                for _ in range(warmup):
                    nc.tensor.matmul(dps[:, 0:512], lhsT, rhs,
                                     start=True, stop=True)

            # ---- quantize (greedy engine balance) -------------------------
            q_rate_b = dict(q_rate)
            q_load_b = dict(q_load)
            if pool_quant == "burst":
                q_rate_b["p"] = 1.48
                q_load_b["p"] = 0.0

            def quant(dst_ap, src_ap, scale_ap, elems, burst=False):
                rates = q_rate_b if burst else q_rate
                loads = q_load_b if burst else q_load
                e = min(loads,
                        key=lambda k: loads[k] + elems * rates[k])
                loads[e] += elems * rates[e] + q_fix[e]
                if e == "v":
                    nc.vector.tensor_scalar(dst_ap, src_ap, scale_ap, None,
                                            op0=mybir.AluOpType.mult)
                elif e == "a":
                    nc.scalar.activation(dst_ap, src_ap,
                                         mybir.ActivationFunctionType.Copy,
                                         scale=scale_ap)
                else:
                    nc.gpsimd.tensor_scalar(dst_ap, src_ap, scale_ap, None,
                                            op0=mybir.AluOpType.mult)

            w8 = [w8pool.tile([128, 2 * NSH], F8, tag="w8", name=f"w8_{kb}")
                  for kb in range(KB)]
            x8 = {}
            for s in range(NSTRIPES):
                for kb in range(KB):
                    x8[(s, kb)] = x8pool.tile([128, 2 * SWM], F8, tag="x8",
                                              name=f"x8_{s}_{kb}")

            def w_src(kb):
                t = kb // 4            # wstg tile (KW k each, 8 chunks)
                c = 2 * kb - 8 * t
                return wstg[t][:, c:c + 2, :].rearrange("p a b -> p (a b)")

            def x_src(s, kb):
                t = kb // 4            # xcov/xstg tile (KW k, 8 chunks)
                c = 2 * kb - 8 * t
                return xstg[(s, t)][:, c:c + 2, :].rearrange(
                    "p a b -> p (a b)")

            # first burst: interleave w8 and x8 stripe-0 in kb order so the
            # PE can accumulate (w8[kb], x8[0,kb]) pairs as they appear
            for kb in range(KB):
                quant(w8[kb][:], w_src(kb), sw, 2 * NSH, burst=True)
                quant(x8[(0, kb)][:], x_src(0, kb), sx, 2 * SWM, burst=True)
            q_load["v"] += q_load_b["v"]
            q_load["a"] += q_load_b["a"]
            if preload:
                # deq+bias land on DVE (and out dispatch on its engine)
                # during the stripe phase; bias the remaining quant splits
                q_load["v"] += 29500.0 if deq == "dve" else 19000.0
                if out_eng == "act":
                    q_load["a"] += 10000.0
                if pool_quant and out_eng == "pool":
                    q_load["p"] += 16000.0

            # ---- matmul sweep ---------------------------------------------
            SPC = MCH // NSTRIPES
            for mc in range(MCH):
                s = mc // SPC
                lm = (mc % SPC) * 128
                sq = (mc + qlead) // SPC   # stripe whose quants to issue now
                if (mc + qlead) % SPC == 0 and 0 < sq < NSTRIPES:
                    for kb in range(KB):
                        quant(x8[(sq, kb)][:], x_src(sq, kb), sx, 2 * SWM)
                ps = ppool.tile([128, NSH], F32, tag="ps")
                if double_row:
                    for kb in range(KB):
                        lhsT = x8[(s, kb)].rearrange(
                            "p (i m) -> p i m", i=2)[:, :, lm:lm + 128]
                        rhs = w8[kb].rearrange("p (i n) -> p i n", i=2)
                        # accumulation group must stay within one 2KB PSUM
                        # bank (512 fp32): run the two n-halves separately
                        for nh in range(2):
                            nc.tensor.matmul(
                                ps[:, nh * (NSH // 2):(nh + 1) * (NSH // 2)],
                                lhsT, rhs[:, :, nh * (NSH // 2):
                                          (nh + 1) * (NSH // 2)],
                                start=(kb == 0), stop=(kb == KB - 1),
                                perf_mode=mybir.MatmulPerfMode.DoubleRow)
                else:
                    for kb in range(KB):
                        for i in range(2):
                            lhsT = x8[(s, kb)][:, i * SWM + lm:
                                               i * SWM + lm + 128]
                            rhs = w8[kb][:, i * NSH:(i + 1) * NSH]
                            nc.tensor.matmul(
                                ps[:], lhsT, rhs,
                                start=(kb == 0 and i == 0),
                                stop=(kb == KB - 1 and i == 1))
                ot = opool.tile([128, NSH], F16, tag="ot")
                if deq == "split":
                    nc.vector.tensor_scalar(ot[:, 0:NSH // 2],
                                            ps[:, 0:NSH // 2], r2[:], None,
                                            op0=mybir.AluOpType.mult)
                    nc.scalar.activation(ot[:, NSH // 2:], ps[:, NSH // 2:],
                                         mybir.ActivationFunctionType.Copy,
                                         scale=r2[:])
                elif deq == "dve" or (deq == "alt" and mc % 2 == 1):
                    nc.vector.tensor_scalar(ot[:], ps[:], r2[:], None,
                                            op0=mybir.AluOpType.mult)
                else:
                    nc.scalar.activation(ot[:], ps[:],
                                         mybir.ActivationFunctionType.Copy,
                                         scale=r2[:])
                nc.vector.tensor_tensor(ot[:], ot[:], bias_b[:],
                                        op=mybir.AluOpType.add)
                if out_eng == "act":
                    nc.scalar.dma_start(out[mc * 128:(mc + 1) * 128, :],
                                        ot[:])
                elif out_eng == "pool":
                    nc.gpsimd.dma_start(out[mc * 128:(mc + 1) * 128, :],
                                        ot[:])
                else:
                    nc.sync.dma_start(out[mc * 128:(mc + 1) * 128, :], ot[:])

    nc.compile()
    return nc


_CACHE = {}


def _get_kernel(M=4096, K=4096, NSH=None, SW=None, double_row=None):
    """NSH/SW args accepted for compatibility; config is fixed internally."""
    key = (M, K)
    if key not in _CACHE:
        _CACHE[key] = build_kernel(M, K, NSH=K // CGRP,
                                   double_row=DOUBLE_ROW)
    return _CACHE[key]


def kernel(x, weight, bias):
    M, K = x.shape
    N = weight.shape[0]
    nc = _get_kernel(M, K)
    MH, NSH = M // RGRP, N // CGRP
    SH = MH // CGRP           # x m-roll unit (x coverage distinctness)
    NR = NSH // RGRP          # w n-roll unit (w coverage distinctness)

    x = np.asarray(x)
    weight = np.asarray(weight)
    bias = np.asarray(bias)
    in_maps = []
    for core in range(NCORES):
        r, c = divmod(core, CGRP)
        xh = np.roll(x[r * MH:(r + 1) * MH], -SH * c, axis=0)
        wq = np.roll(weight[c * NSH:(c + 1) * NSH], -NR * r, axis=0)
        bq = np.roll(bias[c * NSH:(c + 1) * NSH], -NR * r)
        in_maps.append({
            "x": np.ascontiguousarray(xh),
            "w": np.ascontiguousarray(wq),
            "bias": np.ascontiguousarray(bq.reshape(1, NSH)),
        })
    # The axon terminal occasionally reports a stale NRT_EXEC_UNIT error from
    # a previous session on first use; a retry lands on a recovered device.
    last_err = None
    for _ in range(3):
        try:
            res = run_bass_kernel_spmd(nc, in_maps,
                                       core_ids=list(range(NCORES)))
            break
        except Exception as e:  # noqa: BLE001
            last_err = e
            time.sleep(2.0)
    else:
        raise last_err
    full = np.empty((M, N), dtype=np.float16)
    for core in range(NCORES):
        r, c = divmod(core, CGRP)
        o = np.asarray(res.results[core]["out"])
        o = np.roll(o, (SH * c, NR * r), axis=(0, 1))
        full[r * MH:(r + 1) * MH, c * NSH:(c + 1) * NSH] = o
    return full


# revision 45
# speedup vs baseline: 1.0268x; 1.0031x over previous
"""FP8 dynamic-quantized linear (nn_FP8Linear) on 8 Trainium2 NeuronCores.

out = fp16((x_fp8 @ w_fp8.T) / (sx*sw)) + bias, with per-tensor dynamic
fp8-e4m3 quantization of x and weight (scale = FP8_MAX / amax).

Sharding: 2x4 tensor-parallel grid. x rows split in 2 halves (replicated
across the 4 cores of a row group); weight/bias split in 4 column slabs
(replicated across the 2 cores of a column group). Each core computes a
[M/2, N/4] output slab; the host stitches the 8 slabs (no output
collective needed). This cuts per-core fp16 loads to 24MB vs 36MB for
out_features-only sharding.

Global per-tensor amaxes (must match the reference exactly) come from a
"coverage" scheme: each core's FIRST-loaded 8MB -- a distinct quarter of
its x half (m-stripe 0 after a host-side np.roll of the rows) and a
distinct n-half of its w slab (after a host-side n-roll) -- is
abs-max-reduced as it lands in SBUF, split between the DVE and GpSimd
engines so the reduction keeps pace with the DMA. Partials land in
columns of shared accumulators (one final reduce, no combine tree). The
8 cores' partial pairs are exchanged with one tiny AllGather (15us
modeled vs 28us for AllReduce) plus a local max; the union of the 8
coverage sets is exactly x and w, so the scales are the exact global
ones and quantization matches the reference bit-for-bit (modulo the
power-of-2 trick below). The rolls also let every core run the SAME
SPMD program; the host un-rolls the output slab.

The Tile scheduler serializes DmaTranspose against collectives (they
share the DMA/XBAR path), so w is loaded in NATURAL layout (plain DMA
overlaps the collective) and transposed to k-major on the otherwise-
idle PE (matmul-transpose against an identity, fp16 through PSUM is
exact), with psum->SBUF assembly copies on DVE/Act. x coverage is
DMA-transposed before the collective; the x remainder is DMA-transposed
after the scale readback (explicit dep) so it cannot delay the
collective, and output writes are dispatched from the Pool engine so
they never head-of-line-block the SP transpose stream.

Matmuls are fp8 DoubleRow (2x PE rate, 256-deep contraction per pass);
each accumulation group is split into 512-column halves because a
matmul accumulation group must stay inside one 2KB PSUM bank (the
walrus codegen rejects wider groups). Discarded fp16 matmuls bridge the
PE p-state through the amax/collective window.

TRN fp8e4 (float8_e4m3) has max +-240 vs OCP e4m3fn's +-448, so the
device uses scale 224/amax == ref_scale/2: fp8 grids are self-similar
under powers of two, so device fp8 values are exactly half the
reference's, and the dequant multipliers absorb the factor of 4.

Modeled (TimelineSim) exec time: 145370 ns vs 279277 ns for the
previous out_features-sharded kernel (1.92x).
"""

import time

import numpy as np

import concourse.bacc as bacc
import concourse.bass as bass
import concourse.bass_isa as bass_isa
import concourse.mybir as mybir
import concourse.tile as tile
from concourse import masks
from concourse.bass import _add_dep_helper
from concourse.bass_utils import run_bass_kernel_spmd

F16 = mybir.dt.float16
F32 = mybir.dt.float32
F8 = mybir.dt.float8e4

NCORES = 8
RGRP, CGRP = 2, 4       # row groups (x halves) x col groups (w slabs)
EPS = 1e-12
# device-side quantization scale numerator: ref uses 448 (e4m3fn max); we use
# 224 so quantized values stay within TRN e4m3's +-240 normal range.
DEV_FP8_MAX = 224.0
DOUBLE_ROW = True
POOL_QUANT = False
WARMUP = 30


def build_kernel(M=4096, K=4096, NSH=1024, double_row=True,
                 pool_quant=POOL_QUANT, warmup=WARMUP, out_eng="pool",
                 deq="dve", cp_act=False, preload=True, qlead=1):
    """Build + compile the per-core bass program.

    Per-core shapes: x [M/2, K], w [NSH, K], out [M/2, NSH] with NSH=N/4.
    double_row: fp8 DoubleRow matmuls (2x PE throughput, ~1e-4 rel noise).
    warmup: number of discarded fp16 matmuls (gated on the last w load)
    bridging the PE p-state between the w transposes and the fp8 burst.
    pool_quant: also use the gpsimd (Pool) engine for fp16->fp8 quantize.
    """
    MH = M // RGRP            # 2048 token rows per core
    KB = K // 256             # 16 k-blocks (DoubleRow contracts 256/pass)
    NSTRIPES = 4
    SWM = MH // NSTRIPES      # 512-row m-stripes
    MCH = MH // 128           # 16 m-chunks per core
    KW = K // 4               # transfer k-width (1024)
    KCH = K // 128            # 32 k-chunks
    WNT = NSH // 128          # 8 natural w tiles
    assert MH % NSTRIPES == 0 and K % 256 == 0

    nc = bacc.Bacc("TRN2", target_bir_lowering=False, debug=False,
                   num_devices=NCORES)
    x = nc.dram_tensor("x", [MH, K], F16, kind="ExternalInput").ap()
    w = nc.dram_tensor("w", [NSH, K], F16, kind="ExternalInput").ap()
    bias = nc.dram_tensor("bias", [1, NSH], F16, kind="ExternalInput").ap()
    out = nc.dram_tensor("out", [MH, NSH], F16, kind="ExternalOutput").ap()

    # greedy engine balancers (ns/elem/partition + fixed overhead),
    # calibrated against observed TimelineSim slice durations
    cp_rate = {"v": 2.2 if cp_act else 0.72, "a": 1.0}  # psum->SBUF copies
    cp_load = {k: 0.0 for k in cp_rate}
    q_rate = {"v": 0.52, "a": 0.92}               # fp16->fp8 quantize
    if pool_quant:
        q_rate["p"] = 1.48
    q_fix = {"v": 60.0, "a": 150.0, "p": 150.0}
    # reserve DVE for dequant+bias, Act for out-DMA dispatch, Pool for smalls
    q_load = {"v": 0.0, "a": 0.0}
    if pool_quant:
        q_load["p"] = 0.0

    DVE_SHARE = 0.45          # coverage amax: DVE share vs gpsimd

    with tile.TileContext(nc) as tc:
        with (
            tc.tile_pool(name="const", bufs=1) as cpool,
            tc.tile_pool(name="redu", bufs=16) as rpool,
            tc.tile_pool(name="nat", bufs=6) as natpool,
            tc.tile_pool(name="wstg", bufs=4) as wspool,
            tc.tile_pool(name="xstg", bufs=6) as xspool,
            tc.tile_pool(name="w8", bufs=KB) as w8pool,
            tc.tile_pool(name="x8", bufs=KB + 2) as x8pool,
            tc.tile_pool(name="psum", bufs=3, space="PSUM") as ppool,
            tc.tile_pool(name="tp", bufs=2, space="PSUM") as tppool,
            tc.tile_pool(name="ot", bufs=4) as opool,
            tc.tile_pool(name="dram", bufs=2, space="DRAM") as dpool,
        ):
            # ---- constants ------------------------------------------------
            bias_row = cpool.tile([1, NSH], F16, tag="bias_row")
            nc.gpsimd.dma_start(bias_row[:], bias[:])
            bias_b = cpool.tile([128, NSH], F16, tag="bias_b")
            nc.gpsimd.partition_broadcast(bias_b[:], bias_row[:])
            ident = cpool.tile([128, 128], F16, tag="ident")
            masks.make_identity(nc, ident[:])

            # partial amaxes land in columns of shared accumulators; one
            # final reduce replaces a pairwise combine tree
            dax = rpool.tile([128, 8], F32, tag="dax")
            daw = rpool.tile([128, 8], F32, tag="daw")
            pax = rpool.tile([1, 8], F32, tag="pax")
            paw = rpool.tile([1, 8], F32, tag="paw")
            nc.gpsimd.memset(dax[:], 0.0)
            nc.gpsimd.memset(daw[:], 0.0)
            nc.gpsimd.memset(pax[:], 0.0)
            nc.gpsimd.memset(paw[:], 0.0)
            n_d = {"x": 0, "w": 0}

            def amax_of(flat_ap, free, tag):
                h = int(free * DVE_SHARE) & ~63
                da = dax if tag == "x" else daw
                pa = pax if tag == "x" else paw
                i = n_d[tag]
                n_d[tag] += 1
                nc.vector.tensor_reduce(
                    da[:, i:i + 1], flat_ap[:, 0:h],
                    axis=mybir.AxisListType.X,
                    op=mybir.AluOpType.max, apply_absolute_value=True)
                nc.gpsimd.tensor_reduce(
                    pa[:, i:i + 1], flat_ap[:, h:free],
                    axis=mybir.AxisListType.XYZWC,
                    op=mybir.AluOpType.max, apply_absolute_value=True)

            # ---- w natural loads + PE transposes into k-major wstg --------
            # Half-tiles [128 n, K/2] keep the load->transpose->reuse chain
            # fine-grained so DMA never waits on the PE. After the host
            # n-roll, tiles nt<4 are this core's distinct amax coverage.
            def cp(dst_ap, src_ap, elems):
                e = min(cp_load,
                        key=lambda k: cp_load[k] + elems * cp_rate[k])
                cp_load[e] += elems * cp_rate[e] + 250.0
                if e == "v":
                    nc.vector.tensor_copy(dst_ap, src_ap)
                else:
                    nc.scalar.activation(dst_ap, src_ap,
                                         mybir.ActivationFunctionType.Copy)

            wstg = [wspool.tile([128, 8, NSH], F16, tag="wstg",
                                name=f"wstg_{g}") for g in range(4)]
            wnat = {}

            def load_wnat(nt, h):
                nat = natpool.tile([128, K // 2], F16, tag="nat",
                                   name=f"wnat_{nt}_{h}")
                nc.sync.dma_start(
                    nat[:], w[nt * 128:(nt + 1) * 128,
                              h * (K // 2):(h + 1) * (K // 2)])
                wnat[(nt, h)] = nat
                if nt < 4:
                    amax_of(nat[:], K // 2, "w")
                for g in range(2):
                    pst = tppool.tile([128, 8, 128], F16, tag="tp",
                                      name=f"tp_{nt}_{h}_{g}")
                    for j in range(8):
                        c = 8 * g + j
                        nc.tensor.transpose(
                            pst[:, j, :], nat[:, c * 128:(c + 1) * 128],
                            ident[:])
                    cp(wstg[2 * h + g][:, 0:8, nt * 128:(nt + 1) * 128],
                       pst[:], 8 * 128)

            for nt in range(4):
                for h in range(2):
                    load_wnat(nt, h)

            # ---- x stripe-0 coverage: natural half-tiles + PE transpose ---
            # (plain DMA keeps the collective window free of DmaTranspose)
            xstg = {}
            for t in range(4):
                xstg[(0, t)] = xspool.tile([128, KW // 128, SWM], F16,
                                           tag="xstg", name=f"xcov_{t}")
            for mt in range(SWM // 128):
                for h in range(2):
                    nat = natpool.tile([128, K // 2], F16, tag="nat",
                                       name=f"xnat_{mt}_{h}")
                    nc.sync.dma_start(
                        nat[:], x[mt * 128:(mt + 1) * 128,
                                  h * (K // 2):(h + 1) * (K // 2)])
                    amax_of(nat[:], K // 2, "x")
                    for g in range(2):
                        pst = tppool.tile([128, 8, 128], F16, tag="tp",
                                          name=f"xtp_{mt}_{h}_{g}")
                        for j in range(8):
                            c = 8 * g + j
                            nc.tensor.transpose(
                                pst[:, j, :], nat[:, c * 128:(c + 1) * 128],
                                ident[:])
                        cp(xstg[(0, 2 * h + g)][:, 0:8,
                                                mt * 128:(mt + 1) * 128],
                           pst[:], 8 * 128)

            # ---- w rest (overlaps the collective: plain DMA) --------------
            for nt in range(4, WNT):
                for h in range(2):
                    load_wnat(nt, h)

            # ---- AllGather(concat) global amaxes --------------------------
            _hp = tc.high_priority()
            _hp.__enter__()
            amax2 = rpool.tile([128, 2], F32, tag="amax2")
            nc.vector.tensor_reduce(amax2[:, 0:1], dax[:],
                                    axis=mybir.AxisListType.X,
                                    op=mybir.AluOpType.max)
            nc.vector.tensor_reduce(amax2[:, 1:2], daw[:],
                                    axis=mybir.AxisListType.X,
                                    op=mybir.AluOpType.max)
            amax2r = rpool.tile([128, 2], F32, tag="amax2r")
            nc.gpsimd.partition_all_reduce(
                amax2r[:], amax2[:], channels=128,
                reduce_op=bass_isa.ReduceOp.max)
            p2 = rpool.tile([1, 2], F32, tag="p2")
            nc.vector.tensor_reduce(p2[:, 0:1], pax[:],
                                    axis=mybir.AxisListType.X,
                                    op=mybir.AluOpType.max)
            nc.vector.tensor_reduce(p2[:, 1:2], paw[:],
                                    axis=mybir.AxisListType.X,
                                    op=mybir.AluOpType.max)
            bin2 = rpool.tile([1, 2], F32, tag="bin2")
            nc.vector.tensor_tensor(bin2[:], amax2r[0:1, :], p2[:],
                                    op=mybir.AluOpType.max)

            bin_ = dpool.tile([1, 2], F32, name="bin_")
            bout = dpool.tile([1, 2 * NCORES], F32, name="bout")
            nc.gpsimd.dma_start(bin_[:], bin2[:])
            cc = nc.gpsimd.collective_compute(
                "AllGather", mybir.AluOpType.bypass,
                replica_groups=[list(range(NCORES))],
                ins=[bin_.opt()], outs=[bout.opt()])
            g16 = rpool.tile([1, 2 * NCORES], F32, tag="g16")
            g16_read = nc.gpsimd.dma_start(g16[:], bout[:])
            # gathered layout: [c0x, c0w, c1x, c1w, ...] -> max over cores
            gm = rpool.tile([1, 2], F32, tag="gm")
            nc.vector.tensor_reduce(
                gm[:], g16[:].rearrange("a (g t) -> a t g", t=2),
                axis=mybir.AxisListType.X, op=mybir.AluOpType.max)
            nc.vector.tensor_scalar_max(gm[:], gm[:], EPS)
            gb = rpool.tile([128, 2], F32, tag="gb")
            nc.gpsimd.partition_broadcast(gb[:], gm[:])

            # scales: s = 224/amax (quant), r = 1/s (dequant), r2 = rx*rw
            u2 = rpool.tile([128, 2], F32, tag="u2")
            nc.vector.reciprocal(u2[:], gb[:])
            s2 = rpool.tile([128, 2], F32, tag="s2")
            nc.vector.tensor_scalar_mul(s2[:], u2[:], DEV_FP8_MAX)
            inv2 = rpool.tile([128, 2], F32, tag="inv2")
            nc.vector.reciprocal(inv2[:], s2[:])
            r2 = rpool.tile([128, 1], F32, tag="r2")
            nc.vector.tensor_tensor(r2[:], inv2[:, 0:1], inv2[:, 1:2],
                                    op=mybir.AluOpType.mult)
            sx, sw = s2[:, 0:1], s2[:, 1:2]
            _hp.__exit__(None, None, None)

            # ---- stripe-1 k-half 0: natural loads + PE transpose ----------
            # (plain DMA fills the collective-window DMA idle; only tiles
            # (1,0)/(1,1) have free staging bufs this early)
            for t in range(2):
                xstg[(1, t)] = xspool.tile([128, KW // 128, SWM], F16,
                                           tag="xstg", name=f"xstg_1_{t}")
            for mt in range(SWM // 128):
                nat = natpool.tile([128, K // 2], F16, tag="nat",
                                   name=f"x1nat_{mt}")
                nc.sync.dma_start(
                    nat[:], x[SWM + mt * 128:SWM + (mt + 1) * 128,
                              0:K // 2])
                for g in range(2):
                    pst = tppool.tile([128, 8, 128], F16, tag="tp",
                                      name=f"x1tp_{mt}_{g}")
                    for j in range(8):
                        c = 8 * g + j
                        nc.tensor.transpose(
                            pst[:, j, :], nat[:, c * 128:(c + 1) * 128],
                            ident[:])
                    cp(xstg[(1, g)][:, 0:8, mt * 128:(mt + 1) * 128],
                       pst[:], 8 * 128)

            # ---- x rest: transposed loads AFTER the readback --------------
            # (DmaTranspose serializes against the collective; gating these
            # on the readback keeps the collective + scales path clean.)
            for s in range(1, NSTRIPES):
                for q in range(4):
                    if s == 1 and q < 2:
                        continue
                    stg = xspool.tile([128, KW // 128, SWM], F16,
                                      tag="xstg", name=f"xstg_{s}_{q}")
                    d = nc.sync.dma_start(
                        stg[:], x[s * SWM:(s + 1) * SWM,
                                  q * KW:(q + 1) * KW],
                        transpose=True)
                    _add_dep_helper(d.ins, cc.ins, sync=True,
                                    reason="hold transposes off collective")
                    xstg[(s, q)] = stg

            # ---- PE p-state bridge: discarded fp16 matmuls ----------------
            if warmup:
                dps = ppool.tile([128, NSH], F32, tag="ps", name="dps")
                rhs = wnat[(WNT - 1, 1)][:, 0:512]
                lhsT = wnat[(WNT - 1, 1)][:, 512:640]
                for _ in range(warmup):
                    nc.tensor.matmul(dps[:, 0:512], lhsT, rhs,
                                     start=True, stop=True)

            # ---- quantize (greedy engine balance) -------------------------
            q_rate_b = dict(q_rate)
            q_load_b = dict(q_load)
            if pool_quant == "burst":
                q_rate_b["p"] = 1.48
                q_load_b["p"] = 0.0

            def quant(dst_ap, src_ap, scale_ap, elems, burst=False):
                rates = q_rate_b if burst else q_rate
                loads = q_load_b if burst else q_load
                e = min(loads,
                        key=lambda k: loads[k] + elems * rates[k])
                loads[e] += elems * rates[e] + q_fix[e]
                if e == "v":
                    nc.vector.tensor_scalar(dst_ap, src_ap, scale_ap, None,
                                            op0=mybir.AluOpType.mult)
                elif e == "a":
                    nc.scalar.activation(dst_ap, src_ap,
                                         mybir.ActivationFunctionType.Copy,
                                         scale=scale_ap)
                else:
                    nc.gpsimd.tensor_scalar(dst_ap, src_ap, scale_ap, None,
                                            op0=mybir.AluOpType.mult)

            w8 = [w8pool.tile([128, 2 * NSH], F8, tag="w8", name=f"w8_{kb}")
                  for kb in range(KB)]
            x8 = {}
            for s in range(NSTRIPES):
                for kb in range(KB):
                    x8[(s, kb)] = x8pool.tile([128, 2 * SWM], F8, tag="x8",
                                              name=f"x8_{s}_{kb}")

            def w_src(kb):
                t = kb // 4            # wstg tile (KW k each, 8 chunks)
                c = 2 * kb - 8 * t
                return wstg[t][:, c:c + 2, :].rearrange("p a b -> p (a b)")

            def x_src(s, kb):
                t = kb // 4            # xcov/xstg tile (KW k, 8 chunks)
                c = 2 * kb - 8 * t
                return xstg[(s, t)][:, c:c + 2, :].rearrange(
                    "p a b -> p (a b)")

            # first burst: interleave w8 and x8 stripe-0 in kb order so the
            # PE can accumulate (w8[kb], x8[0,kb]) pairs as they appear
            for kb in range(KB):
                quant(x8[(0, kb)][:], x_src(0, kb), sx, 2 * SWM, burst=True)
                quant(w8[kb][:], w_src(kb), sw, 2 * NSH, burst=True)
            q_load["v"] += q_load_b["v"]
            q_load["a"] += q_load_b["a"]
            if preload:
                # deq+bias land on DVE (and out dispatch on its engine)
                # during the stripe phase; bias the remaining quant splits
                q_load["v"] += 29500.0 if deq == "dve" else 19000.0
                if out_eng == "act":
                    q_load["a"] += 10000.0
                if pool_quant and out_eng == "pool":
                    q_load["p"] += 16000.0

            # ---- matmul sweep ---------------------------------------------
            SPC = MCH // NSTRIPES
            for mc in range(MCH):
                s = mc // SPC
                lm = (mc % SPC) * 128
                sq = (mc + qlead) // SPC   # stripe whose quants to issue now
                if (mc + qlead) % SPC == 0 and 0 < sq < NSTRIPES:
                    for kb in range(KB):
                        quant(x8[(sq, kb)][:], x_src(sq, kb), sx, 2 * SWM)
                ps = ppool.tile([128, NSH], F32, tag="ps")
                if double_row:
                    for kb in range(KB):
                        lhsT = x8[(s, kb)].rearrange(
                            "p (i m) -> p i m", i=2)[:, :, lm:lm + 128]
                        rhs = w8[kb].rearrange("p (i n) -> p i n", i=2)
                        # accumulation group must stay within one 2KB PSUM
                        # bank (512 fp32): run the two n-halves separately
                        for nh in range(2):
                            nc.tensor.matmul(
                                ps[:, nh * (NSH // 2):(nh + 1) * (NSH // 2)],
                                lhsT, rhs[:, :, nh * (NSH // 2):
                                          (nh + 1) * (NSH // 2)],
                                start=(kb == 0), stop=(kb == KB - 1),
                                perf_mode=mybir.MatmulPerfMode.DoubleRow)
                else:
                    for kb in range(KB):
                        for i in range(2):
                            lhsT = x8[(s, kb)][:, i * SWM + lm:
                                               i * SWM + lm + 128]
                            rhs = w8[kb][:, i * NSH:(i + 1) * NSH]
                            nc.tensor.matmul(
                                ps[:], lhsT, rhs,
                                start=(kb == 0 and i == 0),
                                stop=(kb == KB - 1 and i == 1))
                ot = opool.tile([128, NSH], F16, tag="ot")
                if deq == "split":
                    nc.vector.tensor_scalar(ot[:, 0:NSH // 2],
                                            ps[:, 0:NSH // 2], r2[:], None,
                                            op0=mybir.AluOpType.mult)
                    nc.scalar.activation(ot[:, NSH // 2:], ps[:, NSH // 2:],
                                         mybir.ActivationFunctionType.Copy,
                                         scale=r2[:])
                elif deq == "dve" or (deq == "alt" and mc % 2 == 1):
                    nc.vector.tensor_scalar(ot[:], ps[:], r2[:], None,
                                            op0=mybir.AluOpType.mult)
                else:
                    nc.scalar.activation(ot[:], ps[:],
                                         mybir.ActivationFunctionType.Copy,
                                         scale=r2[:])
                nc.vector.tensor_tensor(ot[:], ot[:], bias_b[:],
                                        op=mybir.AluOpType.add)
                if out_eng == "act":
                    nc.scalar.dma_start(out[mc * 128:(mc + 1) * 128, :],
                                        ot[:])
                elif out_eng == "pool":
                    nc.gpsimd.dma_start(out[mc * 128:(mc + 1) * 128, :],
                                        ot[:])
                else:
                    nc.sync.dma_start(out[mc * 128:(mc + 1) * 128, :], ot[:])

    nc.compile()
    return nc


_CACHE = {}


def _get_kernel(M=4096, K=4096, NSH=None, SW=None, double_row=None):
    """NSH/SW args accepted for compatibility; config is fixed internally."""
    key = (M, K)
    if key not in _CACHE:
        _CACHE[key] = build_kernel(M, K, NSH=K // CGRP,
                                   double_row=DOUBLE_ROW)
    return _CACHE[key]


def kernel(x, weight, bias):
    M, K = x.shape
    N = weight.shape[0]
    nc = _get_kernel(M, K)
    MH, NSH = M // RGRP, N // CGRP
    SH = MH // CGRP           # x m-roll unit (x coverage distinctness)
    NR = NSH // RGRP          # w n-roll unit (w coverage distinctness)

    x = np.asarray(x)
    weight = np.asarray(weight)
    bias = np.asarray(bias)
    in_maps = []
    for core in range(NCORES):
        r, c = divmod(core, CGRP)
        xh = np.roll(x[r * MH:(r + 1) * MH], -SH * c, axis=0)
        wq = np.roll(weight[c * NSH:(c + 1) * NSH], -NR * r, axis=0)
        bq = np.roll(bias[c * NSH:(c + 1) * NSH], -NR * r)
        in_maps.append({
            "x": np.ascontiguousarray(xh),
            "w": np.ascontiguousarray(wq),
            "bias": np.ascontiguousarray(bq.reshape(1, NSH)),
        })
    # The axon terminal occasionally reports a stale NRT_EXEC_UNIT error from
    # a previous session on first use; a retry lands on a recovered device.
    last_err = None
    for _ in range(3):
        try:
            res = run_bass_kernel_spmd(nc, in_maps,
                                       core_ids=list(range(NCORES)))
            break
        except Exception as e:  # noqa: BLE001
            last_err = e
            time.sleep(2.0)
    else:
        raise last_err
    full = np.empty((M, N), dtype=np.float16)
    for core in range(NCORES):
        r, c = divmod(core, CGRP)
        o = np.asarray(res.results[core]["out"])
        o = np.roll(o, (SH * c, NR * r), axis=(0, 1))
        full[r * MH:(r + 1) * MH, c * NSH:(c + 1) * NSH] = o
    return full
